# revision 17
# baseline (speedup 1.0000x reference)
"""Trainium2 Bass kernel for DualEdgeGraphConvBlock (gnn_message_passing).

Sharding: 8 NeuronCores, SPMD. Core c = b*4 + s handles batch b, query rows
[s*1024, (s+1)*1024). All per-core differences are data-driven (host-sliced
inputs), so a single program runs on all cores.

Key ideas:
  - The reference's fp32 distance matrices are reproduced bit-exactly:
    inner products on PE (fp32 matmul), norms via DVE reduce on natural
    layout, assembly ordered as fl(2e - fl(n_i + n_j)) == -d_ref.
    Top-k then uses DVE max8/find_index8/match_replace, which matches
    jax top_k tie semantics (stable, lowest-index-first) exactly, so the
    idx2 output and all gather lists match the reference bit-for-bit.
  - 1x1 convs are hoisted through the edge gather:
    W @ concat(nbr-ctr, ctr) = (Wa @ f)[nbr] + ((Wb-Wa) @ f)[ctr].
    Node-level maps are computed once on PE, edge values gathered from DRAM
    by indirect DMA and transposed on PE to channels-major, so BN stats and
    k-max pooling are cheap per-partition ops.
  - BN batch stats all-reduced over all 8 cores; h1 all-gathered within each
    batch's 4-core group. LeakyReLU and the BN affine commute with k-max
    (positive scale), so they are applied after the reduction.
"""

import sys

sys.path.insert(0, "/opt/trn_rl_repo")

import numpy as np

import concourse.bass as bass
import concourse.tile as tile
from concourse import bacc, mybir
from concourse.bass import IndirectOffsetOnAxis
from concourse.bass_utils import run_bass_kernel_spmd
from concourse.masks import make_identity

F32 = mybir.dt.float32
BF16 = mybir.dt.bfloat16
U32 = mybir.dt.uint32
I32 = mybir.dt.int32
AX = mybir.AxisListType
OP = mybir.AluOpType
ACTF = mybir.ActivationFunctionType

B, N, C, D = 2, 4096, 64, 3
NS = 1024           # rows per core
QT = NS // 128      # 8 query tiles per core
PT = N // 128       # 32 point tiles
K1X, K1P, K2 = 8, 6, 32
K1 = K1X + K1P      # 14
C1 = 2 * C          # 128
HID = 128
OUTC = 64
EPS = 1e-5
SLOPE = 0.2
NEG = -1.0e30
M1 = float(B * N * K1)
M2 = float(B * N * K2)
NCORES = 8
TRACE = False       # set kernel.TRACE = True before calling for an NTFF profile

_CACHE = {}


def _lrelu_inplace(nc, ap):
    # lrelu(x) = max(0.2*x, x), exact for slope in (0,1)
    nc.vector.scalar_tensor_tensor(out=ap, in0=ap, scalar=SLOPE, in1=ap,
                                   op0=OP.mult, op1=OP.max)


def _bn_coeffs(nc, pool, stat, gb, abn, M, tagp):
    """stat[:,0]=sum, stat[:,1]=sumsq (globally reduced) -> abn = [a, d]."""
    P = stat.shape[0]
    mu = pool.tile([P, 1], F32, tag=tagp + "mu")
    var = pool.tile([P, 1], F32, tag=tagp + "var")
    nc.vector.tensor_scalar_mul(mu[:], stat[:, 0:1], 1.0 / M)
    nc.vector.tensor_scalar_mul(var[:], stat[:, 1:2], 1.0 / M)
    mu2 = pool.tile([P, 1], F32, tag=tagp + "mu2")
    nc.vector.tensor_mul(mu2[:], mu[:], mu[:])
    nc.vector.tensor_sub(var[:], var[:], mu2[:])
    nc.vector.tensor_scalar_add(var[:], var[:], EPS)
    sd = pool.tile([P, 1], F32, tag=tagp + "sd")
    nc.scalar.activation(sd[:], var[:], ACTF.Sqrt)
    inv = pool.tile([P, 1], F32, tag=tagp + "inv")
    nc.vector.reciprocal(inv[:], sd[:])
    nc.vector.tensor_mul(abn[:, 0:1], gb[:, 0:1], inv[:])       # a = g/sd
    tmp = pool.tile([P, 1], F32, tag=tagp + "tmp")
    nc.vector.tensor_mul(tmp[:], mu[:], abn[:, 0:1])
    nc.vector.tensor_sub(abn[:, 1:2], gb[:, 1:2], tmp[:])       # d = b - mu*a


def _bcast_mid(ap2d, k):
    # (128, n) -> (128, k, n) with a step-0 broadcast middle dim
    return ap2d.rearrange("p (a n) -> p a n", a=1).to_broadcast(
        [ap2d.shape[0], k, ap2d.shape[1]])


def _build():
    nc = bacc.Bacc("TRN2", target_bir_lowering=False, debug=False,
                   num_devices=NCORES)

    t_xT = nc.dram_tensor("xT_full", [C, N], F32, kind="ExternalInput")
    t_pT = nc.dram_tensor("posT_full", [D, N], F32, kind="ExternalInput")
    t_xTl = nc.dram_tensor("xT_local", [C, NS], F32, kind="ExternalInput")
    t_pTl = nc.dram_tensor("posT_local", [D, NS], F32, kind="ExternalInput")
    t_xn = nc.dram_tensor("x_nat", [N, C], F32, kind="ExternalInput")
    t_pn = nc.dram_tensor("pos_nat", [N, D], F32, kind="ExternalInput")
    t_xnl = nc.dram_tensor("x_nat_local", [NS, C], F32, kind="ExternalInput")
    t_pnl = nc.dram_tensor("pos_nat_local", [NS, D], F32, kind="ExternalInput")
    t_u1 = nc.dram_tensor("U1T", [C, C1], F32, kind="ExternalInput")
    t_v1 = nc.dram_tensor("V1T", [C, C1], F32, kind="ExternalInput")
    t_u2 = nc.dram_tensor("U2T", [C1, HID], F32, kind="ExternalInput")
    t_v2 = nc.dram_tensor("V2T", [C1, HID], F32, kind="ExternalInput")
    t_w3 = nc.dram_tensor("W3T", [HID, OUTC], F32, kind="ExternalInput")
    t_gb1 = nc.dram_tensor("gb1", [C1, 2], F32, kind="ExternalInput")
    t_gb2 = nc.dram_tensor("gb2", [HID, 2], F32, kind="ExternalInput")
    t_gb3 = nc.dram_tensor("gb3", [OUTC, 2], F32, kind="ExternalInput")

    t_out = nc.dram_tensor("out_slice", [NS, OUTC], F32, kind="ExternalOutput")
    t_idx2 = nc.dram_tensor("idx2_slice", [NS, K2], I32, kind="ExternalOutput")

    io = dict(locals())
    with tile.TileContext(nc) as tc:
        _program(nc, tc, io)
    nc.compile()
    return nc


def _program(nc, tc, T):
    with (
        tc.tile_pool(name="persist", bufs=1) as persist,
        tc.tile_pool(name="dram", bufs=1, space="DRAM") as dram,
    ):
        ident = persist.tile([128, 128], F32)
        make_identity(nc, ident[:])
        idx2_all = persist.tile([128, QT, K2], U32)
        h1_loc = persist.tile([C1, NS], F32)

        A_dram = dram.tile([N, C1], F32)
        P_dram = dram.tile([N, HID], F32)
        nrow_x_d = dram.tile([1, N], F32)
        nrow_p_d = dram.tile([1, N], F32)
        ar1_in = dram.tile([C1, 2], F32)
        ar1_out = dram.tile([C1, 2], F32)
        ar2_in = dram.tile([HID, 2], F32)
        ar2_out = dram.tile([HID, 2], F32)
        ar3_in = dram.tile([OUTC, 2], F32)
        ar3_out = dram.tile([OUTC, 2], F32)
        ag_in = dram.tile([C1, NS], F32)
        ag_out = dram.tile([4, C1, NS], F32)

        _phase12(nc, tc, T, ident, idx2_all, h1_loc, A_dram,
                 nrow_x_d, nrow_p_d, ar1_in, ar1_out)
        _phase3(nc, tc, T, ident, idx2_all, h1_loc, P_dram,
                ag_in, ag_out, ar2_in, ar2_out, ar3_in, ar3_out)


def _phase12(nc, tc, T, ident, idx2_all, h1_loc, A_dram,
             nrow_x_d, nrow_p_d, ar1_in, ar1_out):
    with (
        tc.tile_pool(name="ph1", bufs=1) as p1,
        tc.tile_pool(name="ph1s", bufs=2) as p1s,
        tc.tile_pool(name="ph2", bufs=2) as p2,
        tc.tile_pool(name="stp", bufs=1) as stp,
    ):
        # packed transposed inputs: rows 0:64 = xT, rows 64:67 = posT
        xTpk = p1.tile([128, N], F32)
        xTlpk = p1.tile([128, NS], F32)
        nc.sync.dma_start(xTpk[0:C, :], T["t_xT"].ap())
        nc.sync.dma_start(xTpk[C:C + D, :], T["t_pT"].ap())
        nc.sync.dma_start(xTlpk[0:C, :], T["t_xTl"].ap())
        nc.sync.dma_start(xTlpk[C:C + D, :], T["t_pTl"].ap())

        # weights
        U1 = p1.tile([C, C1], F32)
        V1 = p1.tile([C, C1], F32)
        gb1 = p1.tile([C1, 2], F32)
        nc.sync.dma_start(U1[:], T["t_u1"].ap())
        nc.sync.dma_start(V1[:], T["t_v1"].ap())
        nc.sync.dma_start(gb1[:], T["t_gb1"].ap())

        # ---- norms (bit-exact: square + DVE reduce on natural layout) ----
        ncol_x = p1.tile([128, PT], F32)
        ncol_p = p1.tile([128, PT], F32)
        ni_x = p1.tile([128, QT], F32)
        ni_p = p1.tile([128, QT], F32)
        for i in range(PT):
            xt = p1s.tile([128, C], F32, tag="nx")
            nc.sync.dma_start(xt[:], T["t_xn"].ap()[i * 128:(i + 1) * 128, :])
            sq = p1s.tile([128, C], F32, tag="nsq")
            nc.vector.tensor_mul(sq[:], xt[:], xt[:])
            nc.vector.tensor_reduce(ncol_x[:, i:i + 1], sq[:], axis=AX.X,
                                    op=OP.add)
            pt_ = p1s.tile([128, D], F32, tag="np")
            nc.sync.dma_start(pt_[:], T["t_pn"].ap()[i * 128:(i + 1) * 128, :])
            sqp = p1s.tile([128, D], F32, tag="npsq")
            nc.vector.tensor_mul(sqp[:], pt_[:], pt_[:])
            nc.vector.tensor_reduce(ncol_p[:, i:i + 1], sqp[:], axis=AX.X,
                                    op=OP.add)
        for t in range(QT):
            xt = p1s.tile([128, C], F32, tag="nx")
            nc.sync.dma_start(xt[:], T["t_xnl"].ap()[t * 128:(t + 1) * 128, :])
            sq = p1s.tile([128, C], F32, tag="nsq")
            nc.vector.tensor_mul(sq[:], xt[:], xt[:])
            nc.vector.tensor_reduce(ni_x[:, t:t + 1], sq[:], axis=AX.X,
                                    op=OP.add)
            pt_ = p1s.tile([128, D], F32, tag="np")
            nc.sync.dma_start(pt_[:], T["t_pnl"].ap()[t * 128:(t + 1) * 128, :])
            sqp = p1s.tile([128, D], F32, tag="npsq")
            nc.vector.tensor_mul(sqp[:], pt_[:], pt_[:])
            nc.vector.tensor_reduce(ni_p[:, t:t + 1], sqp[:], axis=AX.X,
                                    op=OP.add)

        # (128, PT) -> flat (1, N) -> broadcast to 128 partitions
        nc.sync.dma_start(
            nrow_x_d[:].rearrange("a (t p) -> p (a t)", p=128), ncol_x[:])
        nc.sync.dma_start(
            nrow_p_d[:].rearrange("a (t p) -> p (a t)", p=128), ncol_p[:])
        nrow_x = p1.tile([1, N], F32)
        nrow_p = p1.tile([1, N], F32)
        nc.sync.dma_start(nrow_x[:], nrow_x_d[:])
        nc.sync.dma_start(nrow_p[:], nrow_p_d[:])
        Sx = p1.tile([128, N], F32)
        Sp = p1.tile([128, N], F32)
        nc.gpsimd.partition_broadcast(Sx[:], nrow_x[0:1, :])
        nc.gpsimd.partition_broadcast(Sp[:], nrow_p[0:1, :])

        # ---- node maps: A^T (row-major, DRAM) and B (channels-major) ----
        B_cm = p1.tile([C1, NS], F32)
        with (
            tc.tile_pool(name="aps", bufs=4, space="PSUM") as aps,
            tc.tile_pool(name="asb", bufs=4) as asb,
        ):
            for i in range(PT):
                ps = aps.tile([128, C1], F32, tag="aps")
                nc.tensor.matmul(ps[:], xTpk[0:C, i * 128:(i + 1) * 128],
                                 U1[:], start=True, stop=True)
                sb = asb.tile([128, C1], F32, tag="asb")
                nc.scalar.copy(sb[:], ps[:])
                nc.sync.dma_start(A_dram[:][i * 128:(i + 1) * 128, :], sb[:])
            for j in range(NS // 512):
                ps = aps.tile([C1, 512], F32, tag="bps")
                nc.tensor.matmul(ps[:], V1[:],
                                 xTlpk[0:C, j * 512:(j + 1) * 512],
                                 start=True, stop=True)
                nc.scalar.copy(B_cm[:, j * 512:(j + 1) * 512], ps[:])

        # ---- per-qtile: distances, top-k, layer-1 edge conv ----
        y1_all = p1.tile([128, QT, K1, 128], BF16)
        psum_s = p1.tile([C1, QT], F32)
        psum_q = p1.tile([C1, QT], F32)
        dnx = p1.tile([128, N], F32)
        dnp = p1.tile([128, N], F32)

        qtile_ps = tc.tile_pool(name="ph1ps", bufs=2, space="PSUM")
        g1ps_cm = tc.tile_pool(name="g1ps", bufs=1, space="PSUM")
        p1ps = qtile_ps.__enter__()
        g1ps = g1ps_cm.__enter__()
        for t in range(QT):
            # negated distances dneg = fl(2e - fl(n_i + n_j)) == -d_ref
            for q in range(4):
                cs = slice(q * 1024, (q + 1) * 1024)
                sxc = p1s.tile([128, 1024], F32, tag="sxc")
                nc.gpsimd.tensor_scalar_add(sxc[:], Sx[:, cs],
                                            ni_x[:, t:t + 1])
                ps = p1ps.tile([128, 1024], F32, tag="eps")
                for h in range(2):
                    c0 = q * 1024 + h * 512
                    nc.tensor.matmul(ps[:, h * 512:(h + 1) * 512],
                                     xTlpk[0:C, t * 128:(t + 1) * 128],
                                     xTpk[0:C, c0:c0 + 512],
                                     start=True, stop=True)
                nc.vector.scalar_tensor_tensor(
                    out=dnx[:, cs], in0=ps[:], scalar=2.0, in1=sxc[:],
                    op0=OP.mult, op1=OP.subtract)
                spc = p1s.tile([128, 1024], F32, tag="spc")
                nc.gpsimd.tensor_scalar_add(spc[:], Sp[:, cs],
                                            ni_p[:, t:t + 1])
                ps2 = p1ps.tile([128, 1024], F32, tag="eps")
                for h in range(2):
                    c0 = q * 1024 + h * 512
                    nc.tensor.matmul(ps2[:, h * 512:(h + 1) * 512],
                                     xTlpk[C:C + D, t * 128:(t + 1) * 128],
                                     xTpk[C:C + D, c0:c0 + 512],
                                     start=True, stop=True)
                nc.vector.scalar_tensor_tensor(
                    out=dnp[:, cs], in0=ps2[:], scalar=2.0, in1=spc[:],
                    op0=OP.mult, op1=OP.subtract)

            # top-k: per-chunk max8 candidates, then full-row index finds
            candx = p1.tile([128, 256], F32, tag="candx")
            candp = p1.tile([128, 256], F32, tag="candp")
            for ch in range(32):
                nc.vector.max(candx[:, ch * 8:(ch + 1) * 8],
                              dnx[:, ch * 128:(ch + 1) * 128])
            for ch in range(32):
                nc.vector.max(candp[:, ch * 8:(ch + 1) * 8],
                              dnp[:, ch * 128:(ch + 1) * 128])
            vx = p1.tile([128, 16], F32, tag="vx")
            fx = p1.tile([128, 16], U32, tag="fx")
            nc.vector.max(vx[:, 0:8], candx[:])
            nc.vector.max_index(fx[:, 0:8], vx[:, 0:8], dnx[:])
            nc.vector.match_replace(candx[:], vx[:, 0:8], candx[:], NEG)
            nc.vector.max(vx[:, 8:16], candx[:])
            nc.vector.max_index(fx[:, 8:16], vx[:, 8:16], dnx[:])

            vp = p1.tile([128, 40], F32, tag="vp")
            fp = p1.tile([128, 40], U32, tag="fp")
            for r in range(5):
                sl = slice(r * 8, (r + 1) * 8)
                nc.vector.max(vp[:, sl], candp[:])
                nc.vector.max_index(fp[:, sl], vp[:, sl], dnp[:])
                if r < 4:
                    nc.vector.match_replace(dnp[:], vp[:, sl], dnp[:], NEG)
                    nc.vector.match_replace(candp[:], vp[:, sl], candp[:],
                                            NEG)

            nc.vector.tensor_copy(idx2_all[:, t, :], fp[:, 1:33])
            idx2_i32 = p1.tile([128, K2], I32, tag="idx2i")
            nc.vector.tensor_copy(idx2_i32[:], fp[:, 1:33])
            nc.sync.dma_start(
                T["t_idx2"].ap()[t * 128:(t + 1) * 128, :], idx2_i32[:])

            # layer 1: gather A rows per k, transpose to channels-major, +B
            g1 = p2.tile([128, K1, C1], F32, tag="g1")
            for k in range(K1):
                off = fx[:, 1 + k:2 + k] if k < K1X else fp[:, k - 7:k - 6]
                nc.gpsimd.indirect_dma_start(
                    out=g1[:, k, :], out_offset=None, in_=A_dram[:],
                    in_offset=IndirectOffsetOnAxis(ap=off, axis=0))
            yps = g1ps.tile([128, K1 * 128], F32, tag="yps")
            for k in range(K1):
                nc.tensor.transpose(yps[:, k * 128:(k + 1) * 128],
                                    g1[:, k, :], ident[:])
            nc.vector.scalar_tensor_tensor(
                out=y1_all[:, t, :, :],
                in0=yps[:].rearrange("p (k n) -> p k n", k=K1),
                scalar=0.0, op0=OP.add,
                in1=_bcast_mid(B_cm[:, t * 128:(t + 1) * 128], K1),
                op1=OP.add,
                accum_out=psum_s[:, t:t + 1])
            scr = p2.tile([128, K1 * 128], BF16, tag="scr1")
            nc.scalar.activation(
                scr[:], y1_all[:, t, :, :].rearrange("p a b -> p (a b)"),
                ACTF.Square, accum_out=psum_q[:, t:t + 1])

        g1ps_cm.__exit__(None, None, None)
        qtile_ps.__exit__(None, None, None)

        # ---- BN1 stats allreduce; h1 = lrelu(a*kmax(y1)+d) ----
        stat1 = stp.tile([C1, 2], F32)
        nc.vector.tensor_reduce(stat1[:, 0:1], psum_s[:], axis=AX.X, op=OP.add)
        nc.vector.tensor_reduce(stat1[:, 1:2], psum_q[:], axis=AX.X, op=OP.add)
        nc.sync.dma_start(ar1_in[:], stat1[:])
        nc.gpsimd.collective_compute(
            "AllReduce", OP.add, replica_groups=[list(range(NCORES))],
            ins=[ar1_in.opt()], outs=[ar1_out.opt()])
        nc.sync.dma_start(stat1[:], ar1_out[:])
        abn1 = stp.tile([C1, 2], F32)
        _bn_coeffs(nc, stp, stat1, gb1, abn1, M1, "b1")

        for t in range(QT):
            mx = p1s.tile([128, 128], F32, tag="mx1")
            nc.vector.tensor_reduce(
                mx[:], y1_all[:, t, :, :].rearrange("p k n -> p n k"),
                axis=AX.X, op=OP.max)
            nc.scalar.activation(h1_loc[:, t * 128:(t + 1) * 128], mx[:],
                                 ACTF.Identity, bias=abn1[:, 1:2],
                                 scale=abn1[:, 0:1])
            _lrelu_inplace(nc, h1_loc[:, t * 128:(t + 1) * 128])


def _phase3(nc, tc, T, ident, idx2_all, h1_loc, P_dram,
            ag_in, ag_out, ar2_in, ar2_out, ar3_in, ar3_out):
    with (
        tc.tile_pool(name="ph3", bufs=1) as p3,
        tc.tile_pool(name="st3", bufs=1) as stp,
    ):
        # allgather h1 within the 4-core batch group
        nc.sync.dma_start(ag_in[:], h1_loc[:])
        nc.gpsimd.collective_compute(
            "AllGather", OP.bypass,
            replica_groups=[[0, 1, 2, 3], [4, 5, 6, 7]],
            ins=[ag_in.opt()], outs=[ag_out.opt()])

        U2 = p3.tile([C1, HID], F32)
        V2 = p3.tile([C1, HID], F32)
        W3 = p3.tile([HID, OUTC], F32)
        gb2 = p3.tile([HID, 2], F32)
        gb3 = p3.tile([OUTC, 2], F32)
        nc.sync.dma_start(U2[:], T["t_u2"].ap())
        nc.sync.dma_start(V2[:], T["t_v2"].ap())
        nc.sync.dma_start(W3[:], T["t_w3"].ap())
        nc.sync.dma_start(gb2[:], T["t_gb2"].ap())
        nc.sync.dma_start(gb3[:], T["t_gb3"].ap())

        y2b = p3.tile([HID, QT, K2, 128], BF16)
        s2s = p3.tile([HID, QT * 2], F32)
        s2q = p3.tile([HID, QT * 2], F32)

        with (
            tc.tile_pool(name="ph3a", bufs=1) as pa,
            tc.tile_pool(name="ph3a2", bufs=2) as pa2,
        ):
            h1f = pa.tile([C1, N], F32)
            for s in range(4):
                nc.sync.dma_start(h1f[:, s * NS:(s + 1) * NS],
                                  ag_out[:][s, :, :])
            # P map rows to DRAM; Q map channels-major (local rows)
            Q_cm = pa.tile([HID, NS], F32)
            with tc.tile_pool(name="pps", bufs=4, space="PSUM") as pps:
                for i in range(PT):
                    ps = pps.tile([128, HID], F32, tag="pps")
                    nc.tensor.matmul(ps[:], h1f[:, i * 128:(i + 1) * 128],
                                     U2[:], start=True, stop=True)
                    sb = pa2.tile([128, HID], F32, tag="psb")
                    nc.scalar.copy(sb[:], ps[:])
                    nc.sync.dma_start(P_dram[:][i * 128:(i + 1) * 128, :],
                                      sb[:])
                for j in range(NS // 512):
                    ps = pps.tile([HID, 512], F32, tag="qps")
                    nc.tensor.matmul(ps[:], V2[:],
                                     h1_loc[:, j * 512:(j + 1) * 512],
                                     start=True, stop=True)
                    nc.scalar.copy(Q_cm[:, j * 512:(j + 1) * 512], ps[:])

            # layer 2 per qtile: gather P rows, transpose, +Q -> y2 (bf16)
            g2ps_cm = tc.tile_pool(name="g2ps", bufs=2, space="PSUM")
            g2ps = g2ps_cm.__enter__()
            for t in range(QT):
                g2 = pa2.tile([128, K2, HID], F32, tag="g2")
                for k in range(K2):
                    nc.gpsimd.indirect_dma_start(
                        out=g2[:, k, :], out_offset=None, in_=P_dram[:],
                        in_offset=IndirectOffsetOnAxis(
                            ap=idx2_all[:, t, k:k + 1], axis=0))
                for half in range(2):
                    ps = g2ps.tile([128, 16 * 128], F32, tag="g2ps")
                    for k in range(16):
                        kk = half * 16 + k
                        nc.tensor.transpose(ps[:, k * 128:(k + 1) * 128],
                                            g2[:, kk, :], ident[:])
                    col = t * 2 + half
                    nc.vector.scalar_tensor_tensor(
                        out=y2b[:, t, half * 16:(half + 1) * 16, :],
                        in0=ps[:].rearrange("p (k n) -> p k n", k=16),
                        scalar=0.0, op0=OP.add,
                        in1=_bcast_mid(Q_cm[:, t * 128:(t + 1) * 128], 16),
                        op1=OP.add,
                        accum_out=s2s[:, col:col + 1])
                    scr = pa2.tile([128, 16 * 128], BF16, tag="scr2")
                    nc.scalar.activation(
                        scr[:],
                        y2b[:, t, half * 16:(half + 1) * 16, :].rearrange(
                            "p a b -> p (a b)"),
                        ACTF.Square, accum_out=s2q[:, col:col + 1])
            g2ps_cm.__exit__(None, None, None)

        stat2 = stp.tile([HID, 2], F32)
        nc.vector.tensor_reduce(stat2[:, 0:1], s2s[:], axis=AX.X, op=OP.add)
        nc.vector.tensor_reduce(stat2[:, 1:2], s2q[:], axis=AX.X, op=OP.add)
        nc.sync.dma_start(ar2_in[:], stat2[:])
        nc.gpsimd.collective_compute(
            "AllReduce", OP.add, replica_groups=[list(range(NCORES))],
            ins=[ar2_in.opt()], outs=[ar2_out.opt()])
        nc.sync.dma_start(stat2[:], ar2_out[:])
        abn2 = stp.tile([HID, 2], F32)
        _bn_coeffs(nc, stp, stat2, gb2, abn2, M2, "b2")

        # ---- layer 3: h2 = lrelu(a2*y2+d2); y3 = W3 @ h2; stats + k-max ----
        s3s = stp.tile([128, QT // 2], F32)
        s3q = stp.tile([128, QT // 2], F32)
        mx_all = stp.tile([128, (QT // 2) * 128], F32)
        with (
            tc.tile_pool(name="ph3b", bufs=2) as pb,
            tc.tile_pool(name="y3ps", bufs=4, space="PSUM") as y3ps,
        ):
            for pair in range(QT // 2):
                h2s = []
                for half in range(2):
                    t = pair * 2 + half
                    h2 = pb.tile([HID, K2 * 128], F32, tag=f"h2{half}")
                    nc.scalar.activation(
                        h2[:],
                        y2b[:, t, :, :].rearrange("p a b -> p (a b)"),
                        ACTF.Identity, bias=abn2[:, 1:2], scale=abn2[:, 0:1])
                    _lrelu_inplace(nc, h2[:])
                    h2s.append(h2)
                y3 = pb.tile([128, K2 * 128], F32, tag="y3")
                for q in range(K2 * 128 // 512):
                    cs = slice(q * 512, (q + 1) * 512)
                    ps = y3ps.tile([128, 512], F32, tag="y3ps")
                    nc.tensor.matmul(ps[0:OUTC, :], W3[:], h2s[0][:, cs],
                                     start=True, stop=True)
                    nc.tensor.matmul(ps[OUTC:128, :], W3[:], h2s[1][:, cs],
                                     start=True, stop=True)
                    nc.scalar.copy(y3[:, cs], ps[:])
                col = slice(pair, pair + 1)
                nc.vector.tensor_reduce(s3s[:, col], y3[:], axis=AX.X,
                                        op=OP.add)
                scr = pb.tile([128, K2 * 128], BF16, tag="scr3")
                nc.scalar.activation(scr[:], y3[:], ACTF.Square,
                                     accum_out=s3q[:, col])
                nc.vector.tensor_reduce(
                    mx_all[:, pair * 128:(pair + 1) * 128],
                    y3[:].rearrange("p (k n) -> p n k", k=K2),
                    axis=AX.X, op=OP.max)

        # fold packed halves (cross-partition via DMA), allreduce, coeffs
        s3s_f = stp.tile([128, 1], F32)
        s3q_f = stp.tile([128, 1], F32)
        nc.vector.tensor_reduce(s3s_f[:], s3s[:], axis=AX.X, op=OP.add)
        nc.vector.tensor_reduce(s3q_f[:], s3q[:], axis=AX.X, op=OP.add)
        hi = stp.tile([OUTC, 2], F32)
        nc.sync.dma_start(hi[:, 0:1], s3s_f[OUTC:128, :])
        nc.sync.dma_start(hi[:, 1:2], s3q_f[OUTC:128, :])
        stat3 = stp.tile([OUTC, 2], F32)
        nc.vector.tensor_add(stat3[:, 0:1], s3s_f[0:OUTC, :], hi[:, 0:1])
        nc.vector.tensor_add(stat3[:, 1:2], s3q_f[0:OUTC, :], hi[:, 1:2])
        nc.sync.dma_start(ar3_in[:], stat3[:])
        nc.gpsimd.collective_compute(
            "AllReduce", OP.add, replica_groups=[list(range(NCORES))],
            ins=[ar3_in.opt()], outs=[ar3_out.opt()])
        nc.sync.dma_start(stat3[:], ar3_out[:])
        abn3_64 = stp.tile([OUTC, 2], F32)
        _bn_coeffs(nc, stp, stat3, gb3, abn3_64, M2, "b3")
        abn3 = stp.tile([128, 2], F32)
        nc.vector.tensor_copy(abn3[0:OUTC, :], abn3_64[:])
        nc.sync.dma_start(abn3[OUTC:128, :], abn3_64[:])

        # ---- out = lrelu(a3 * kmax(y3) + d3), transpose, store ----
        with (
            tc.tile_pool(name="oute", bufs=2) as po,
            tc.tile_pool(name="outps", bufs=2, space="PSUM") as ops_,
        ):
            for pair in range(QT // 2):
                af = po.tile([128, 128], F32, tag="afo")
                nc.scalar.activation(
                    af[:], mx_all[:, pair * 128:(pair + 1) * 128],
                    ACTF.Identity, bias=abn3[:, 1:2], scale=abn3[:, 0:1])
                _lrelu_inplace(nc, af[:])
                tp = ops_.tile([128, 128], F32, tag="tpo")
                nc.tensor.transpose(tp[:], af[:], ident[:])
                ot = po.tile([128, 128], F32, tag="oto")
                nc.scalar.copy(ot[:], tp[:])
                r0 = pair * 256
                nc.sync.dma_start(T["t_out"].ap()[r0:r0 + 128, :],
                                  ot[:, 0:OUTC])
                nc.sync.dma_start(T["t_out"].ap()[r0 + 128:r0 + 256, :],
                                  ot[:, OUTC:128])


def timed_exec(iters=20):
    """Re-run the last kernel invocation's executable with device-resident
    inputs, timing back-to-back executions. Returns (min_s, mean_s)."""
    import time

    import jax
    from jax.sharding import Mesh, PartitionSpec
    from jax.experimental.shard_map import shard_map

    from concourse import bass2jax, mybir

    nc = _CACHE["nc"]
    in_maps = _CACHE["last_in_maps"]
    n_cores = NCORES

    partition_name = (nc.partition_id_tensor.name
                      if nc.partition_id_tensor else None)
    in_names, out_names, out_avals, zero_outs = [], [], [], []
    for alloc in nc.m.functions[0].allocations:
        if not isinstance(alloc, mybir.MemoryLocationSet):
            continue
        name = alloc.memorylocations[0].name
        if alloc.kind == "ExternalInput":
            if name != partition_name:
                in_names.append(name)
        elif alloc.kind == "ExternalOutput":
            out_names.append(name)
            out_avals.append(jax.core.ShapedArray(
                tuple(alloc.tensor_shape), mybir.dt.np(alloc.dtype)))
            zero_outs.append(np.zeros(tuple(alloc.tensor_shape),
                                      mybir.dt.np(alloc.dtype)))
    n_params = len(in_names)
    all_names = in_names + out_names
    if partition_name is not None:
        all_names.append(partition_name)

    def _body(*args):
        operands = list(args)
        if partition_name is not None:
            operands.append(bass2jax.partition_id_tensor())
        outs = bass2jax._bass_exec_p.bind(
            *operands,
            out_avals=tuple(out_avals),
            in_names=tuple(all_names),
            out_names=tuple(out_names),
            lowering_input_output_aliases=(),
            sim_require_finite=True,
            sim_require_nnan=True,
            nc=nc,
        )
        return tuple(outs)

    devices = jax.devices()[:n_cores]
    mesh = Mesh(np.asarray(devices), ("core",))
    nin = n_params + len(out_names)
    fn = jax.jit(
        shard_map(_body, mesh=mesh,
                  in_specs=(PartitionSpec("core"),) * nin,
                  out_specs=(PartitionSpec("core"),) * len(out_names),
                  check_rep=False),
        keep_unused=True,
    )
    from jax.sharding import NamedSharding
    sh = NamedSharding(mesh, PartitionSpec("core"))
    args = []
    for i, name in enumerate(in_names):
        cat = np.concatenate([np.asarray(m[name]) for m in in_maps], axis=0)
        args.append(jax.device_put(cat, sh))
    for z in zero_outs:
        cat = np.zeros((n_cores * z.shape[0], *z.shape[1:]), z.dtype)
        args.append(jax.device_put(cat, sh))
    # warmup + compile
    outs = fn(*args)
    jax.block_until_ready(outs)
    times = []
    for _ in range(iters):
        t0 = time.perf_counter()
        outs = fn(*args)
        jax.block_until_ready(outs)
        times.append(time.perf_counter() - t0)
    return min(times), sum(times) / len(times)


def kernel(x, pos, W1, g1, b1, W2, g2, b2, W3, g3, b3):
    x = np.ascontiguousarray(np.asarray(x, dtype=np.float32))
    pos = np.ascontiguousarray(np.asarray(pos, dtype=np.float32))
    W1 = np.asarray(W1, np.float32); W2 = np.asarray(W2, np.float32)
    W3 = np.asarray(W3, np.float32)
    g1 = np.asarray(g1, np.float32); b1 = np.asarray(b1, np.float32)
    g2 = np.asarray(g2, np.float32); b2 = np.asarray(b2, np.float32)
    g3 = np.asarray(g3, np.float32); b3 = np.asarray(b3, np.float32)

    if "nc" not in _CACHE:
        _CACHE["nc"] = _build()
    nc = _CACHE["nc"]

    U1T = np.ascontiguousarray(W1[:, :C].T)
    V1T = np.ascontiguousarray((W1[:, C:] - W1[:, :C]).T)
    U2T = np.ascontiguousarray(W2[:, :C1].T)
    V2T = np.ascontiguousarray((W2[:, C1:] - W2[:, :C1]).T)
    W3T = np.ascontiguousarray(W3.T)
    gb1 = np.ascontiguousarray(np.stack([g1, b1], axis=1))
    gb2 = np.ascontiguousarray(np.stack([g2, b2], axis=1))
    gb3 = np.ascontiguousarray(np.stack([g3, b3], axis=1))

    in_maps = []
    for c in range(NCORES):
        b, s = divmod(c, 4)
        sl = slice(s * NS, (s + 1) * NS)
        xT = np.ascontiguousarray(x[b].T)
        pT = np.ascontiguousarray(pos[b].T)
        in_maps.append({
            "xT_full": xT, "posT_full": pT,
            "xT_local": np.ascontiguousarray(xT[:, sl]),
            "posT_local": np.ascontiguousarray(pT[:, sl]),
            "x_nat": x[b], "pos_nat": pos[b],
            "x_nat_local": np.ascontiguousarray(x[b][sl]),
            "pos_nat_local": np.ascontiguousarray(pos[b][sl]),
            "U1T": U1T, "V1T": V1T, "U2T": U2T, "V2T": V2T, "W3T": W3T,
            "gb1": gb1, "gb2": gb2, "gb3": gb3,
        })

    _CACHE["last_in_maps"] = in_maps
    res = run_bass_kernel_spmd(nc, in_maps, core_ids=list(range(NCORES)),
                               trace=TRACE)
    _CACHE["last_results"] = res

    out = np.empty((B, N, OUTC), np.float32)
    idx2 = np.empty((B, N, K2), np.int32)
    for c in range(NCORES):
        b, s = divmod(c, 4)
        sl = slice(s * NS, (s + 1) * NS)
        out[b, sl] = res.results[c]["out_slice"]
        idx2[b, sl] = res.results[c]["idx2_slice"]
    return out, idx2


# revision 29
# speedup vs baseline: 56.7901x; 56.7901x over previous
"""Trainium2 Bass kernel for DualEdgeGraphConvBlock (gnn_message_passing).

Sharding: 8 NeuronCores, SPMD. Core c = b*4 + s handles batch b, query rows
[s*1024, (s+1)*1024). All per-core differences are data-driven (host-sliced
inputs), so a single program runs on all cores.

Key ideas:
  - The reference's fp32 distance matrices are reproduced bit-exactly:
    inner products on PE (fp32 matmul), norms via DVE reduce on natural
    layout, assembly ordered as fl(2e - fl(n_i + n_j)) == -d_ref.
    Top-k then uses DVE max8/find_index8/match_replace, which matches
    jax top_k tie semantics (stable, lowest-index-first) exactly, so the
    idx2 output and all gather lists match the reference bit-for-bit.
  - 1x1 convs are hoisted through the edge gather:
    W @ concat(nbr-ctr, ctr) = (Wa @ f)[nbr] + ((Wb-Wa) @ f)[ctr].
    Node-level maps are computed once on PE, edge values gathered from DRAM
    by indirect DMA and transposed on PE to channels-major, so BN stats and
    k-max pooling are cheap per-partition ops.
  - BN batch stats all-reduced over all 8 cores; h1 all-gathered within each
    batch's 4-core group. LeakyReLU and the BN affine commute with k-max
    (positive scale), so they are applied after the reduction.
"""

import sys

sys.path.insert(0, "/opt/trn_rl_repo")

import numpy as np

import concourse.bass as bass
import concourse.tile as tile
from concourse import bacc, mybir
from concourse.bass import IndirectOffsetOnAxis
from concourse.bass_utils import run_bass_kernel_spmd
from concourse.masks import make_identity

F32 = mybir.dt.float32
BF16 = mybir.dt.bfloat16
U32 = mybir.dt.uint32
I32 = mybir.dt.int32
AX = mybir.AxisListType
OP = mybir.AluOpType
ACTF = mybir.ActivationFunctionType

B, N, C, D = 2, 4096, 64, 3
NS = 1024           # rows per core
QT = NS // 128      # 8 query tiles per core
PT = N // 128       # 32 point tiles
K1X, K1P, K2 = 8, 6, 32
K1 = K1X + K1P      # 14
C1 = 2 * C          # 128
HID = 128
OUTC = 64
EPS = 1e-5
SLOPE = 0.2
NEG = -1.0e30
M1 = float(B * N * K1)
M2 = float(B * N * K2)
NCORES = 8
TRACE = False       # set kernel.TRACE = True before calling for an NTFF profile
STUBS = set()       # timing-attribution stubs: {"coll","gather","topk","dist"}

_CACHE = {}


def _lrelu_inplace(nc, ap):
    # lrelu(x) = max(0.2*x, x), exact for slope in (0,1)
    nc.vector.scalar_tensor_tensor(out=ap, in0=ap, scalar=SLOPE, in1=ap,
                                   op0=OP.mult, op1=OP.max)


def _bn_coeffs(nc, pool, stat, gb, abn, M, tagp):
    """stat[:,0]=sum, stat[:,1]=sumsq (globally reduced) -> abn = [a, d]."""
    P = stat.shape[0]
    mu = pool.tile([P, 1], F32, tag=tagp + "mu")
    var = pool.tile([P, 1], F32, tag=tagp + "var")
    nc.vector.tensor_scalar_mul(mu[:], stat[:, 0:1], 1.0 / M)
    nc.vector.tensor_scalar_mul(var[:], stat[:, 1:2], 1.0 / M)
    mu2 = pool.tile([P, 1], F32, tag=tagp + "mu2")
    nc.vector.tensor_mul(mu2[:], mu[:], mu[:])
    nc.vector.tensor_sub(var[:], var[:], mu2[:])
    nc.vector.tensor_scalar_add(var[:], var[:], EPS)
    sd = pool.tile([P, 1], F32, tag=tagp + "sd")
    nc.scalar.activation(sd[:], var[:], ACTF.Sqrt)
    inv = pool.tile([P, 1], F32, tag=tagp + "inv")
    nc.vector.reciprocal(inv[:], sd[:])
    nc.vector.tensor_mul(abn[:, 0:1], gb[:, 0:1], inv[:])       # a = g/sd
    tmp = pool.tile([P, 1], F32, tag=tagp + "tmp")
    nc.vector.tensor_mul(tmp[:], mu[:], abn[:, 0:1])
    nc.vector.tensor_sub(abn[:, 1:2], gb[:, 1:2], tmp[:])       # d = b - mu*a


def _bcast_mid(ap2d, k):
    # (128, n) -> (128, k, n) with a step-0 broadcast middle dim
    return ap2d.rearrange("p (a n) -> p a n", a=1).to_broadcast(
        [ap2d.shape[0], k, ap2d.shape[1]])


def _build():
    nc = bacc.Bacc("TRN2", target_bir_lowering=False, debug=False,
                   num_devices=NCORES)

    t_xT = nc.dram_tensor("xT_full", [C, N], F32, kind="ExternalInput")
    t_pT = nc.dram_tensor("posT_full", [D, N], F32, kind="ExternalInput")
    t_xTl = nc.dram_tensor("xT_local", [C, NS], F32, kind="ExternalInput")
    t_pTl = nc.dram_tensor("posT_local", [D, NS], F32, kind="ExternalInput")
    t_xn = nc.dram_tensor("x_nat", [N, C], F32, kind="ExternalInput")
    t_pn = nc.dram_tensor("pos_nat", [N, D], F32, kind="ExternalInput")
    t_xnl = nc.dram_tensor("x_nat_local", [NS, C], F32, kind="ExternalInput")
    t_pnl = nc.dram_tensor("pos_nat_local", [NS, D], F32, kind="ExternalInput")
    t_u1 = nc.dram_tensor("U1T", [C, C1], F32, kind="ExternalInput")
    t_v1 = nc.dram_tensor("V1T", [C, C1], F32, kind="ExternalInput")
    t_u2 = nc.dram_tensor("U2T", [C1, HID], F32, kind="ExternalInput")
    t_v2 = nc.dram_tensor("V2T", [C1, HID], F32, kind="ExternalInput")
    t_w3 = nc.dram_tensor("W3T", [HID, OUTC], F32, kind="ExternalInput")
    t_gb1 = nc.dram_tensor("gb1", [C1, 2], F32, kind="ExternalInput")
    t_gb2 = nc.dram_tensor("gb2", [HID, 2], F32, kind="ExternalInput")
    t_gb3 = nc.dram_tensor("gb3", [OUTC, 2], F32, kind="ExternalInput")

    t_out = nc.dram_tensor("out_slice", [NS, OUTC], F32, kind="ExternalOutput")
    t_idx2 = nc.dram_tensor("idx2_slice", [NS, K2], I32, kind="ExternalOutput")

    io = dict(locals())
    with tile.TileContext(nc) as tc:
        _program(nc, tc, io)
    nc.compile()
    return nc


def _program(nc, tc, T):
    with (
        tc.tile_pool(name="persist", bufs=1) as persist,
        tc.tile_pool(name="dram", bufs=1, space="DRAM") as dram,
    ):
        ident = persist.tile([128, 128], F32)
        make_identity(nc, ident[:])
        idx2_all = persist.tile([128, QT, K2], U32)
        h1_loc = persist.tile([C1, NS], F32)

        A_dram = dram.tile([N, C1], F32)
        P_dram = dram.tile([N, HID], F32)
        nrow_x_d = dram.tile([1, N], F32)
        nrow_p_d = dram.tile([1, N], F32)
        ar1_in = dram.tile([C1, 2], F32)
        ar1_out = dram.tile([C1, 2], F32)
        ar2_in = dram.tile([HID, 2], F32)
        ar2_out = dram.tile([HID, 2], F32)
        ar3_in = dram.tile([OUTC, 2], F32)
        ar3_out = dram.tile([OUTC, 2], F32)
        ag_in = dram.tile([C1, NS], F32)
        ag_out = dram.tile([4, C1, NS], F32)

        _phase12(nc, tc, T, ident, idx2_all, h1_loc, A_dram,
                 nrow_x_d, nrow_p_d, ar1_in, ar1_out)
        _phase3(nc, tc, T, ident, idx2_all, h1_loc, P_dram,
                ag_in, ag_out, ar2_in, ar2_out, ar3_in, ar3_out)


def _phase12(nc, tc, T, ident, idx2_all, h1_loc, A_dram,
             nrow_x_d, nrow_p_d, ar1_in, ar1_out):
    with (
        tc.tile_pool(name="ph1", bufs=1) as p1,
        tc.tile_pool(name="ph1s", bufs=2) as p1s,
        tc.tile_pool(name="ph2", bufs=2) as p2,
        tc.tile_pool(name="stp", bufs=1) as stp,
    ):
        # packed transposed inputs: rows 0:64 = xT, rows 64:67 = posT
        xTpk = p1.tile([128, N], F32)
        xTlpk = p1.tile([128, NS], F32)
        nc.sync.dma_start(xTpk[0:C, :], T["t_xT"].ap())
        nc.sync.dma_start(xTpk[C:C + D, :], T["t_pT"].ap())
        nc.sync.dma_start(xTlpk[0:C, :], T["t_xTl"].ap())
        nc.sync.dma_start(xTlpk[C:C + D, :], T["t_pTl"].ap())

        # weights
        U1 = p1.tile([C, C1], F32)
        V1 = p1.tile([C, C1], F32)
        gb1 = p1.tile([C1, 2], F32)
        nc.sync.dma_start(U1[:], T["t_u1"].ap())
        nc.sync.dma_start(V1[:], T["t_v1"].ap())
        nc.sync.dma_start(gb1[:], T["t_gb1"].ap())

        # ---- norms (bit-exact: square + DVE reduce on natural layout) ----
        ncol_x = p1.tile([128, PT], F32)
        ncol_p = p1.tile([128, PT], F32)
        ni_x = p1.tile([128, QT], F32)
        ni_p = p1.tile([128, QT], F32)
        for i in range(PT):
            xt = p1s.tile([128, C], F32, tag="nx")
            nc.sync.dma_start(xt[:], T["t_xn"].ap()[i * 128:(i + 1) * 128, :])
            sq = p1s.tile([128, C], F32, tag="nsq")
            nc.vector.tensor_mul(sq[:], xt[:], xt[:])
            nc.vector.tensor_reduce(ncol_x[:, i:i + 1], sq[:], axis=AX.X,
                                    op=OP.add)
            pt_ = p1s.tile([128, D], F32, tag="np")
            nc.sync.dma_start(pt_[:], T["t_pn"].ap()[i * 128:(i + 1) * 128, :])
            sqp = p1s.tile([128, D], F32, tag="npsq")
            nc.vector.tensor_mul(sqp[:], pt_[:], pt_[:])
            nc.vector.tensor_reduce(ncol_p[:, i:i + 1], sqp[:], axis=AX.X,
                                    op=OP.add)
        for t in range(QT):
            xt = p1s.tile([128, C], F32, tag="nx")
            nc.sync.dma_start(xt[:], T["t_xnl"].ap()[t * 128:(t + 1) * 128, :])
            sq = p1s.tile([128, C], F32, tag="nsq")
            nc.vector.tensor_mul(sq[:], xt[:], xt[:])
            nc.vector.tensor_reduce(ni_x[:, t:t + 1], sq[:], axis=AX.X,
                                    op=OP.add)
            pt_ = p1s.tile([128, D], F32, tag="np")
            nc.sync.dma_start(pt_[:], T["t_pnl"].ap()[t * 128:(t + 1) * 128, :])
            sqp = p1s.tile([128, D], F32, tag="npsq")
            nc.vector.tensor_mul(sqp[:], pt_[:], pt_[:])
            nc.vector.tensor_reduce(ni_p[:, t:t + 1], sqp[:], axis=AX.X,
                                    op=OP.add)

        # (128, PT) -> flat (1, N) -> broadcast to 128 partitions
        nc.sync.dma_start(
            nrow_x_d[:].rearrange("a (t p) -> p (a t)", p=128), ncol_x[:])
        nc.sync.dma_start(
            nrow_p_d[:].rearrange("a (t p) -> p (a t)", p=128), ncol_p[:])
        nrow_x = p1.tile([1, N], F32)
        nrow_p = p1.tile([1, N], F32)
        nc.sync.dma_start(nrow_x[:], nrow_x_d[:])
        nc.sync.dma_start(nrow_p[:], nrow_p_d[:])
        # broadcast rows to 128 partitions via PE ones-matmul (x1.0 is exact;
        # gpsimd partition_broadcast is slow)
        Sx = p1.tile([128, N], F32)
        Sp = p1.tile([128, N], F32)
        ones_col = p1.tile([1, 128], F32)
        nc.gpsimd.memset(ones_col[:], 1.0)
        with tc.tile_pool(name="bcps", bufs=4, space="PSUM") as bcps:
            for j in range(N // 512):
                cs = slice(j * 512, (j + 1) * 512)
                ps = bcps.tile([128, 512], F32, tag="bc")
                nc.tensor.matmul(ps[:], ones_col[:], nrow_x[:, cs],
                                 start=True, stop=True)
                nc.scalar.copy(Sx[:, cs], ps[:])
                ps2 = bcps.tile([128, 512], F32, tag="bc")
                nc.tensor.matmul(ps2[:], ones_col[:], nrow_p[:, cs],
                                 start=True, stop=True)
                nc.scalar.copy(Sp[:, cs], ps2[:])

        # ---- node maps: A^T (row-major, DRAM) and B (channels-major) ----
        B_cm = p1.tile([C1, NS], F32)
        with (
            tc.tile_pool(name="aps", bufs=4, space="PSUM") as aps,
            tc.tile_pool(name="asb", bufs=4) as asb,
        ):
            for i in range(PT):
                ps = aps.tile([128, C1], F32, tag="aps")
                nc.tensor.matmul(ps[:], xTpk[0:C, i * 128:(i + 1) * 128],
                                 U1[:], start=True, stop=True)
                sb = asb.tile([128, C1], F32, tag="asb")
                nc.scalar.copy(sb[:], ps[:])
                nc.sync.dma_start(A_dram[:][i * 128:(i + 1) * 128, :], sb[:])
            for j in range(NS // 512):
                ps = aps.tile([C1, 512], F32, tag="bps")
                nc.tensor.matmul(ps[:], V1[:],
                                 xTlpk[0:C, j * 512:(j + 1) * 512],
                                 start=True, stop=True)
                nc.scalar.copy(B_cm[:, j * 512:(j + 1) * 512], ps[:])

        # ---- per-qtile: distances, top-k, layer-1 edge conv ----
        y1_all = p1.tile([128, QT, K1, 128], BF16)
        psum_s = p1.tile([C1, QT], F32)
        psum_q = p1.tile([C1, QT], F32)
        dnx = p1.tile([128, N], F32)
        dnp = p1.tile([128, N], F32)

        qtile_ps = tc.tile_pool(name="ph1ps", bufs=2, space="PSUM")
        g1ps_cm = tc.tile_pool(name="g1ps", bufs=1, space="PSUM")
        p1ps = qtile_ps.__enter__()
        g1ps = g1ps_cm.__enter__()
        for t in range(QT):
            # negated distances dneg = fl(2e - fl(n_i + n_j)) == -d_ref
            if "dist" in STUBS:
                nc.gpsimd.memset(dnx[:], -1.0)
                nc.gpsimd.memset(dnp[:], -1.0)
            for q in range(0 if "dist" in STUBS else 4):
                cs = slice(q * 1024, (q + 1) * 1024)
                sxc = p1s.tile([128, 1024], F32, tag="sxc")
                nc.vector.tensor_scalar_add(sxc[:], Sx[:, cs],
                                            ni_x[:, t:t + 1])
                ps = p1ps.tile([128, 1024], F32, tag="eps")
                for h in range(2):
                    c0 = q * 1024 + h * 512
                    nc.tensor.matmul(ps[:, h * 512:(h + 1) * 512],
                                     xTlpk[0:C, t * 128:(t + 1) * 128],
                                     xTpk[0:C, c0:c0 + 512],
                                     start=True, stop=True)
                nc.vector.scalar_tensor_tensor(
                    out=dnx[:, cs], in0=ps[:], scalar=2.0, in1=sxc[:],
                    op0=OP.mult, op1=OP.subtract)
                spc = p1s.tile([128, 1024], F32, tag="spc")
                nc.vector.tensor_scalar_add(spc[:], Sp[:, cs],
                                            ni_p[:, t:t + 1])
                ps2 = p1ps.tile([128, 1024], F32, tag="eps")
                for h in range(2):
                    c0 = q * 1024 + h * 512
                    nc.tensor.matmul(ps2[:, h * 512:(h + 1) * 512],
                                     xTlpk[C:C + D, t * 128:(t + 1) * 128],
                                     xTpk[C:C + D, c0:c0 + 512],
                                     start=True, stop=True)
                nc.vector.scalar_tensor_tensor(
                    out=dnp[:, cs], in0=ps2[:], scalar=2.0, in1=spc[:],
                    op0=OP.mult, op1=OP.subtract)

            # top-k: per-chunk max8 candidates, then full-row index finds
            if "topk" in STUBS:
                fx = p1.tile([128, 16], U32, tag="fx")
                fp = p1.tile([128, 40], U32, tag="fp")
                nc.gpsimd.memset(fx[:], 0)
                nc.gpsimd.memset(fp[:], 0)
            candx = p1.tile([128, 256], F32, tag="candx")
            candp = p1.tile([128, 256], F32, tag="candp")
            if "topk" not in STUBS:
              for ch in range(32):
                nc.vector.max(candx[:, ch * 8:(ch + 1) * 8],
                              dnx[:, ch * 128:(ch + 1) * 128])
            if "topk" not in STUBS:
              for ch in range(32):
                nc.vector.max(candp[:, ch * 8:(ch + 1) * 8],
                              dnp[:, ch * 128:(ch + 1) * 128])
              vx = p1.tile([128, 16], F32, tag="vx")
              fx = p1.tile([128, 16], U32, tag="fx")
              nc.vector.max(vx[:, 0:8], candx[:])
              nc.vector.max_index(fx[:, 0:8], vx[:, 0:8], dnx[:])
              nc.vector.match_replace(candx[:], vx[:, 0:8], candx[:], NEG)
              nc.vector.max(vx[:, 8:16], candx[:])
              nc.vector.max_index(fx[:, 8:16], vx[:, 8:16], dnx[:])

              vp = p1.tile([128, 40], F32, tag="vp")
              fp = p1.tile([128, 40], U32, tag="fp")
              for r in range(5):
                sl = slice(r * 8, (r + 1) * 8)
                nc.vector.max(vp[:, sl], candp[:])
                nc.vector.max_index(fp[:, sl], vp[:, sl], dnp[:])
                if r < 4:
                    nc.vector.match_replace(dnp[:], vp[:, sl], dnp[:], NEG)
                    nc.vector.match_replace(candp[:], vp[:, sl], candp[:],
                                            NEG)

            nc.vector.tensor_copy(idx2_all[:, t, :], fp[:, 1:33])
            idx2_i32 = p1.tile([128, K2], I32, tag="idx2i")
            nc.vector.tensor_copy(idx2_i32[:], fp[:, 1:33])
            nc.sync.dma_start(
                T["t_idx2"].ap()[t * 128:(t + 1) * 128, :], idx2_i32[:])

            # layer 1: gather A rows per k, transpose to channels-major, +B
            g1 = p2.tile([128, K1, C1], F32, tag="g1")
            for k in range(K1):
                if "gather" in STUBS:
                    nc.sync.dma_start(g1[:, k, :],
                                      A_dram[:][k * 128:(k + 1) * 128, :])
                    continue
                off = fx[:, 1 + k:2 + k] if k < K1X else fp[:, k - 7:k - 6]
                nc.gpsimd.indirect_dma_start(
                    out=g1[:, k, :], out_offset=None, in_=A_dram[:],
                    in_offset=IndirectOffsetOnAxis(ap=off, axis=0))
            yps = g1ps.tile([128, K1 * 128], F32, tag="yps")
            for k in range(K1):
                nc.tensor.transpose(yps[:, k * 128:(k + 1) * 128],
                                    g1[:, k, :], ident[:])
            nc.vector.scalar_tensor_tensor(
                out=y1_all[:, t, :, :],
                in0=yps[:].rearrange("p (k n) -> p k n", k=K1),
                scalar=0.0, op0=OP.add,
                in1=_bcast_mid(B_cm[:, t * 128:(t + 1) * 128], K1),
                op1=OP.add,
                accum_out=psum_s[:, t:t + 1])
            scr = p2.tile([128, K1 * 128], BF16, tag="scr1")
            nc.scalar.activation(
                scr[:], y1_all[:, t, :, :].rearrange("p a b -> p (a b)"),
                ACTF.Square, accum_out=psum_q[:, t:t + 1])

        g1ps_cm.__exit__(None, None, None)
        qtile_ps.__exit__(None, None, None)

        # ---- BN1 stats allreduce; h1 = lrelu(a*kmax(y1)+d) ----
        stat1 = stp.tile([C1, 2], F32)
        nc.vector.tensor_reduce(stat1[:, 0:1], psum_s[:], axis=AX.X, op=OP.add)
        nc.vector.tensor_reduce(stat1[:, 1:2], psum_q[:], axis=AX.X, op=OP.add)
        nc.sync.dma_start(ar1_in[:], stat1[:])
        if "coll" in STUBS:
            nc.sync.dma_start(ar1_out[:], ar1_in[:])
        else:
            nc.gpsimd.collective_compute(
                "AllReduce", OP.add, replica_groups=[list(range(NCORES))],
                ins=[ar1_in.opt()], outs=[ar1_out.opt()])
        nc.sync.dma_start(stat1[:], ar1_out[:])
        abn1 = stp.tile([C1, 2], F32)
        _bn_coeffs(nc, stp, stat1, gb1, abn1, M1, "b1")

        for t in range(QT):
            mx = p1s.tile([128, 128], F32, tag="mx1")
            nc.vector.tensor_reduce(
                mx[:], y1_all[:, t, :, :].rearrange("p k n -> p n k"),
                axis=AX.X, op=OP.max)
            nc.scalar.activation(h1_loc[:, t * 128:(t + 1) * 128], mx[:],
                                 ACTF.Identity, bias=abn1[:, 1:2],
                                 scale=abn1[:, 0:1])
            _lrelu_inplace(nc, h1_loc[:, t * 128:(t + 1) * 128])


def _phase3(nc, tc, T, ident, idx2_all, h1_loc, P_dram,
            ag_in, ag_out, ar2_in, ar2_out, ar3_in, ar3_out):
    with (
        tc.tile_pool(name="ph3", bufs=1) as p3,
        tc.tile_pool(name="st3", bufs=1) as stp,
    ):
        # allgather h1 within the 4-core batch group
        nc.sync.dma_start(ag_in[:], h1_loc[:])
        if "coll" in STUBS:
            for _s in range(4):
                nc.sync.dma_start(ag_out[:][_s, :, :], ag_in[:])
        else:
            nc.gpsimd.collective_compute(
                "AllGather", OP.bypass,
                replica_groups=[[0, 1, 2, 3], [4, 5, 6, 7]],
                ins=[ag_in.opt()], outs=[ag_out.opt()])

        U2 = p3.tile([C1, HID], F32)
        V2 = p3.tile([C1, HID], F32)
        W3 = p3.tile([HID, OUTC], F32)
        gb2 = p3.tile([HID, 2], F32)
        gb3 = p3.tile([OUTC, 2], F32)
        nc.sync.dma_start(U2[:], T["t_u2"].ap())
        nc.sync.dma_start(V2[:], T["t_v2"].ap())
        nc.sync.dma_start(W3[:], T["t_w3"].ap())
        nc.sync.dma_start(gb2[:], T["t_gb2"].ap())
        nc.sync.dma_start(gb3[:], T["t_gb3"].ap())

        y2b = p3.tile([HID, QT, K2, 128], BF16)
        s2s = p3.tile([HID, QT * 2], F32)
        s2q = p3.tile([HID, QT * 2], F32)

        with (
            tc.tile_pool(name="ph3a", bufs=1) as pa,
            tc.tile_pool(name="ph3a2", bufs=2) as pa2,
        ):
            h1f = pa.tile([C1, N], F32)
            for s in range(4):
                nc.sync.dma_start(h1f[:, s * NS:(s + 1) * NS],
                                  ag_out[:][s, :, :])
            # P map rows to DRAM; Q map channels-major (local rows)
            Q_cm = pa.tile([HID, NS], F32)
            with tc.tile_pool(name="pps", bufs=4, space="PSUM") as pps:
                for i in range(PT):
                    ps = pps.tile([128, HID], F32, tag="pps")
                    nc.tensor.matmul(ps[:], h1f[:, i * 128:(i + 1) * 128],
                                     U2[:], start=True, stop=True)
                    sb = pa2.tile([128, HID], F32, tag="psb")
                    nc.scalar.copy(sb[:], ps[:])
                    nc.sync.dma_start(P_dram[:][i * 128:(i + 1) * 128, :],
                                      sb[:])
                for j in range(NS // 512):
                    ps = pps.tile([HID, 512], F32, tag="qps")
                    nc.tensor.matmul(ps[:], V2[:],
                                     h1_loc[:, j * 512:(j + 1) * 512],
                                     start=True, stop=True)
                    nc.scalar.copy(Q_cm[:, j * 512:(j + 1) * 512], ps[:])

            # layer 2 per qtile: gather P rows, transpose, +Q -> y2 (bf16)
            g2ps_cm = tc.tile_pool(name="g2ps", bufs=2, space="PSUM")
            g2ps = g2ps_cm.__enter__()
            for t in range(QT):
                g2 = pa2.tile([128, K2, HID], F32, tag="g2")
                for k in range(K2):
                    if "gather" in STUBS:
                        nc.sync.dma_start(g2[:, k, :],
                                          P_dram[:][k * 128:(k + 1) * 128, :])
                        continue
                    nc.gpsimd.indirect_dma_start(
                        out=g2[:, k, :], out_offset=None, in_=P_dram[:],
                        in_offset=IndirectOffsetOnAxis(
                            ap=idx2_all[:, t, k:k + 1], axis=0))
                for half in range(2):
                    ps = g2ps.tile([128, 16 * 128], F32, tag="g2ps")
                    for k in range(16):
                        kk = half * 16 + k
                        nc.tensor.transpose(ps[:, k * 128:(k + 1) * 128],
                                            g2[:, kk, :], ident[:])
                    col = t * 2 + half
                    nc.vector.scalar_tensor_tensor(
                        out=y2b[:, t, half * 16:(half + 1) * 16, :],
                        in0=ps[:].rearrange("p (k n) -> p k n", k=16),
                        scalar=0.0, op0=OP.add,
                        in1=_bcast_mid(Q_cm[:, t * 128:(t + 1) * 128], 16),
                        op1=OP.add,
                        accum_out=s2s[:, col:col + 1])
                    scr = pa2.tile([128, 16 * 128], BF16, tag="scr2")
                    nc.scalar.activation(
                        scr[:],
                        y2b[:, t, half * 16:(half + 1) * 16, :].rearrange(
                            "p a b -> p (a b)"),
                        ACTF.Square, accum_out=s2q[:, col:col + 1])
            g2ps_cm.__exit__(None, None, None)

        stat2 = stp.tile([HID, 2], F32)
        nc.vector.tensor_reduce(stat2[:, 0:1], s2s[:], axis=AX.X, op=OP.add)
        nc.vector.tensor_reduce(stat2[:, 1:2], s2q[:], axis=AX.X, op=OP.add)
        nc.sync.dma_start(ar2_in[:], stat2[:])
        if "coll" in STUBS:
            nc.sync.dma_start(ar2_out[:], ar2_in[:])
        else:
            nc.gpsimd.collective_compute(
                "AllReduce", OP.add, replica_groups=[list(range(NCORES))],
                ins=[ar2_in.opt()], outs=[ar2_out.opt()])
        nc.sync.dma_start(stat2[:], ar2_out[:])
        abn2 = stp.tile([HID, 2], F32)
        _bn_coeffs(nc, stp, stat2, gb2, abn2, M2, "b2")

        # ---- layer 3: h2 = lrelu(a2*y2+d2); y3 = W3 @ h2; stats + k-max ----
        s3s = stp.tile([128, QT // 2], F32)
        s3q = stp.tile([128, QT // 2], F32)
        mx_all = stp.tile([128, (QT // 2) * 128], F32)
        with (
            tc.tile_pool(name="ph3b", bufs=2) as pb,
            tc.tile_pool(name="y3ps", bufs=4, space="PSUM") as y3ps,
        ):
            for pair in range(QT // 2):
                h2s = []
                for half in range(2):
                    t = pair * 2 + half
                    h2 = pb.tile([HID, K2 * 128], F32, tag=f"h2{half}")
                    nc.scalar.activation(
                        h2[:],
                        y2b[:, t, :, :].rearrange("p a b -> p (a b)"),
                        ACTF.Identity, bias=abn2[:, 1:2], scale=abn2[:, 0:1])
                    _lrelu_inplace(nc, h2[:])
                    h2s.append(h2)
                y3 = pb.tile([128, K2 * 128], F32, tag="y3")
                nchunk = K2 * 128 // 512
                s3c = pb.tile([128, nchunk], F32, tag="s3c")
                for q in range(nchunk):
                    cs = slice(q * 512, (q + 1) * 512)
                    ps = y3ps.tile([128, 512], F32, tag="y3ps")
                    nc.tensor.matmul(ps[0:OUTC, :], W3[:], h2s[0][:, cs],
                                     start=True, stop=True)
                    nc.tensor.matmul(ps[OUTC:128, :], W3[:], h2s[1][:, cs],
                                     start=True, stop=True)
                    nc.scalar.activation(y3[:, cs], ps[:], ACTF.Copy,
                                         accum_out=s3c[:, q:q + 1])
                col = slice(pair, pair + 1)
                nc.vector.tensor_reduce(s3s[:, col], s3c[:], axis=AX.X,
                                        op=OP.add)
                scr = pb.tile([128, K2 * 128], BF16, tag="scr3")
                nc.scalar.activation(scr[:], y3[:], ACTF.Square,
                                     accum_out=s3q[:, col])
                nc.vector.tensor_reduce(
                    mx_all[:, pair * 128:(pair + 1) * 128],
                    y3[:].rearrange("p (k n) -> p n k", k=K2),
                    axis=AX.X, op=OP.max)

        # fold packed halves (cross-partition via DMA), allreduce, coeffs
        s3s_f = stp.tile([128, 1], F32)
        s3q_f = stp.tile([128, 1], F32)
        nc.vector.tensor_reduce(s3s_f[:], s3s[:], axis=AX.X, op=OP.add)
        nc.vector.tensor_reduce(s3q_f[:], s3q[:], axis=AX.X, op=OP.add)
        hi = stp.tile([OUTC, 2], F32)
        nc.sync.dma_start(hi[:, 0:1], s3s_f[OUTC:128, :])
        nc.sync.dma_start(hi[:, 1:2], s3q_f[OUTC:128, :])
        stat3 = stp.tile([OUTC, 2], F32)
        nc.vector.tensor_add(stat3[:, 0:1], s3s_f[0:OUTC, :], hi[:, 0:1])
        nc.vector.tensor_add(stat3[:, 1:2], s3q_f[0:OUTC, :], hi[:, 1:2])
        nc.sync.dma_start(ar3_in[:], stat3[:])
        if "coll" in STUBS:
            nc.sync.dma_start(ar3_out[:], ar3_in[:])
        else:
            nc.gpsimd.collective_compute(
                "AllReduce", OP.add, replica_groups=[list(range(NCORES))],
                ins=[ar3_in.opt()], outs=[ar3_out.opt()])
        nc.sync.dma_start(stat3[:], ar3_out[:])
        abn3_64 = stp.tile([OUTC, 2], F32)
        _bn_coeffs(nc, stp, stat3, gb3, abn3_64, M2, "b3")
        abn3 = stp.tile([128, 2], F32)
        nc.vector.tensor_copy(abn3[0:OUTC, :], abn3_64[:])
        nc.sync.dma_start(abn3[OUTC:128, :], abn3_64[:])

        # ---- out = lrelu(a3 * kmax(y3) + d3), transpose, store ----
        with (
            tc.tile_pool(name="oute", bufs=2) as po,
            tc.tile_pool(name="outps", bufs=2, space="PSUM") as ops_,
        ):
            for pair in range(QT // 2):
                af = po.tile([128, 128], F32, tag="afo")
                nc.scalar.activation(
                    af[:], mx_all[:, pair * 128:(pair + 1) * 128],
                    ACTF.Identity, bias=abn3[:, 1:2], scale=abn3[:, 0:1])
                _lrelu_inplace(nc, af[:])
                tp = ops_.tile([128, 128], F32, tag="tpo")
                nc.tensor.transpose(tp[:], af[:], ident[:])
                ot = po.tile([128, 128], F32, tag="oto")
                nc.scalar.copy(ot[:], tp[:])
                r0 = pair * 256
                nc.sync.dma_start(T["t_out"].ap()[r0:r0 + 128, :],
                                  ot[:, 0:OUTC])
                nc.sync.dma_start(T["t_out"].ap()[r0 + 128:r0 + 256, :],
                                  ot[:, OUTC:128])


def build_replicated(reps):
    """Build a program with the kernel body replicated `reps` times
    (for device-time measurement by wall-clock delta vs reps=1)."""
    nc = bacc.Bacc("TRN2", target_bir_lowering=False, debug=False,
                   num_devices=NCORES)
    t_xT = nc.dram_tensor("xT_full", [C, N], F32, kind="ExternalInput")
    t_pT = nc.dram_tensor("posT_full", [D, N], F32, kind="ExternalInput")
    t_xTl = nc.dram_tensor("xT_local", [C, NS], F32, kind="ExternalInput")
    t_pTl = nc.dram_tensor("posT_local", [D, NS], F32, kind="ExternalInput")
    t_xn = nc.dram_tensor("x_nat", [N, C], F32, kind="ExternalInput")
    t_pn = nc.dram_tensor("pos_nat", [N, D], F32, kind="ExternalInput")
    t_xnl = nc.dram_tensor("x_nat_local", [NS, C], F32, kind="ExternalInput")
    t_pnl = nc.dram_tensor("pos_nat_local", [NS, D], F32, kind="ExternalInput")
    t_u1 = nc.dram_tensor("U1T", [C, C1], F32, kind="ExternalInput")
    t_v1 = nc.dram_tensor("V1T", [C, C1], F32, kind="ExternalInput")
    t_u2 = nc.dram_tensor("U2T", [C1, HID], F32, kind="ExternalInput")
    t_v2 = nc.dram_tensor("V2T", [C1, HID], F32, kind="ExternalInput")
    t_w3 = nc.dram_tensor("W3T", [HID, OUTC], F32, kind="ExternalInput")
    t_gb1 = nc.dram_tensor("gb1", [C1, 2], F32, kind="ExternalInput")
    t_gb2 = nc.dram_tensor("gb2", [HID, 2], F32, kind="ExternalInput")
    t_gb3 = nc.dram_tensor("gb3", [OUTC, 2], F32, kind="ExternalInput")
    t_out = nc.dram_tensor("out_slice", [NS, OUTC], F32, kind="ExternalOutput")
    t_idx2 = nc.dram_tensor("idx2_slice", [NS, K2], I32, kind="ExternalOutput")
    io = dict(locals())
    with tile.TileContext(nc) as tc:
        for _ in range(reps):
            _program(nc, tc, io)
    nc.compile()
    return nc


def timed_exec(iters=20, nc=None, pipelined=True):
    """Run the kernel executable with device-resident inputs. With
    pipelined=True, dispatch `iters` calls asynchronously and divide the
    total wall by iters (amortizes the ~3.3ms axon relay floor).
    Returns (min_or_piped_s, mean_s)."""
    import time

    import jax
    from jax.sharding import Mesh, PartitionSpec
    from jax.experimental.shard_map import shard_map

    from concourse import bass2jax, mybir

    if nc is None:
        nc = _CACHE["nc"]
    in_maps = _CACHE["last_in_maps"]
    n_cores = NCORES

    partition_name = (nc.partition_id_tensor.name
                      if nc.partition_id_tensor else None)
    in_names, out_names, out_avals, zero_outs = [], [], [], []
    for alloc in nc.m.functions[0].allocations:
        if not isinstance(alloc, mybir.MemoryLocationSet):
            continue
        name = alloc.memorylocations[0].name
        if alloc.kind == "ExternalInput":
            if name != partition_name:
                in_names.append(name)
        elif alloc.kind == "ExternalOutput":
            out_names.append(name)
            out_avals.append(jax.core.ShapedArray(
                tuple(alloc.tensor_shape), mybir.dt.np(alloc.dtype)))
            zero_outs.append(np.zeros(tuple(alloc.tensor_shape),
                                      mybir.dt.np(alloc.dtype)))
    n_params = len(in_names)
    all_names = in_names + out_names
    if partition_name is not None:
        all_names.append(partition_name)

    def _body(*args):
        operands = list(args)
        if partition_name is not None:
            operands.append(bass2jax.partition_id_tensor())
        outs = bass2jax._bass_exec_p.bind(
            *operands,
            out_avals=tuple(out_avals),
            in_names=tuple(all_names),
            out_names=tuple(out_names),
            lowering_input_output_aliases=(),
            sim_require_finite=True,
            sim_require_nnan=True,
            nc=nc,
        )
        return tuple(outs)

    devices = jax.devices()[:n_cores]
    mesh = Mesh(np.asarray(devices), ("core",))
    nin = n_params + len(out_names)
    fn = jax.jit(
        shard_map(_body, mesh=mesh,
                  in_specs=(PartitionSpec("core"),) * nin,
                  out_specs=(PartitionSpec("core"),) * len(out_names),
                  check_rep=False),
        keep_unused=True,
    )
    from jax.sharding import NamedSharding
    sh = NamedSharding(mesh, PartitionSpec("core"))
    args = []
    for i, name in enumerate(in_names):
        cat = np.concatenate([np.asarray(m[name]) for m in in_maps], axis=0)
        args.append(jax.device_put(cat, sh))
    for z in zero_outs:
        cat = np.zeros((n_cores * z.shape[0], *z.shape[1:]), z.dtype)
        args.append(jax.device_put(cat, sh))
    # warmup + compile
    outs = fn(*args)
    jax.block_until_ready(outs)
    if pipelined:
        t0 = time.perf_counter()
        outs_l = [fn(*args) for _ in range(iters)]
        jax.block_until_ready(outs_l)
        piped = (time.perf_counter() - t0) / iters
        return piped, piped
    times = []
    for _ in range(iters):
        t0 = time.perf_counter()
        outs = fn(*args)
        jax.block_until_ready(outs)
        times.append(time.perf_counter() - t0)
    return min(times), sum(times) / len(times)


def kernel(x, pos, W1, g1, b1, W2, g2, b2, W3, g3, b3):
    x = np.ascontiguousarray(np.asarray(x, dtype=np.float32))
    pos = np.ascontiguousarray(np.asarray(pos, dtype=np.float32))
    W1 = np.asarray(W1, np.float32); W2 = np.asarray(W2, np.float32)
    W3 = np.asarray(W3, np.float32)
    g1 = np.asarray(g1, np.float32); b1 = np.asarray(b1, np.float32)
    g2 = np.asarray(g2, np.float32); b2 = np.asarray(b2, np.float32)
    g3 = np.asarray(g3, np.float32); b3 = np.asarray(b3, np.float32)

    if "nc" not in _CACHE:
        _CACHE["nc"] = _build()
    nc = _CACHE["nc"]

    U1T = np.ascontiguousarray(W1[:, :C].T)
    V1T = np.ascontiguousarray((W1[:, C:] - W1[:, :C]).T)
    U2T = np.ascontiguousarray(W2[:, :C1].T)
    V2T = np.ascontiguousarray((W2[:, C1:] - W2[:, :C1]).T)
    W3T = np.ascontiguousarray(W3.T)
    gb1 = np.ascontiguousarray(np.stack([g1, b1], axis=1))
    gb2 = np.ascontiguousarray(np.stack([g2, b2], axis=1))
    gb3 = np.ascontiguousarray(np.stack([g3, b3], axis=1))

    in_maps = []
    for c in range(NCORES):
        b, s = divmod(c, 4)
        sl = slice(s * NS, (s + 1) * NS)
        xT = np.ascontiguousarray(x[b].T)
        pT = np.ascontiguousarray(pos[b].T)
        in_maps.append({
            "xT_full": xT, "posT_full": pT,
            "xT_local": np.ascontiguousarray(xT[:, sl]),
            "posT_local": np.ascontiguousarray(pT[:, sl]),
            "x_nat": x[b], "pos_nat": pos[b],
            "x_nat_local": np.ascontiguousarray(x[b][sl]),
            "pos_nat_local": np.ascontiguousarray(pos[b][sl]),
            "U1T": U1T, "V1T": V1T, "U2T": U2T, "V2T": V2T, "W3T": W3T,
            "gb1": gb1, "gb2": gb2, "gb3": gb3,
        })

    _CACHE["last_in_maps"] = in_maps
    res = run_bass_kernel_spmd(nc, in_maps, core_ids=list(range(NCORES)),
                               trace=TRACE)
    _CACHE["last_results"] = res

    out = np.empty((B, N, OUTC), np.float32)
    idx2 = np.empty((B, N, K2), np.int32)
    for c in range(NCORES):
        b, s = divmod(c, 4)
        sl = slice(s * NS, (s + 1) * NS)
        out[b, sl] = res.results[c]["out_slice"]
        idx2[b, sl] = res.results[c]["idx2_slice"]
    return out, idx2


# revision 31
# speedup vs baseline: 58.8844x; 1.0369x over previous
"""Trainium2 Bass kernel for DualEdgeGraphConvBlock (gnn_message_passing).

Sharding: 8 NeuronCores, SPMD. Core c = b*4 + s handles batch b, query rows
[s*1024, (s+1)*1024). All per-core differences are data-driven (host-sliced
inputs), so a single program runs on all cores.

Key ideas:
  - The reference's fp32 distance matrices are reproduced bit-exactly:
    inner products on PE (fp32 matmul), norms via DVE reduce on natural
    layout, assembly ordered as fl(2e - fl(n_i + n_j)) == -d_ref.
    Top-k then uses DVE max8/find_index8/match_replace, which matches
    jax top_k tie semantics (stable, lowest-index-first) exactly, so the
    idx2 output and all gather lists match the reference bit-for-bit.
  - 1x1 convs are hoisted through the edge gather:
    W @ concat(nbr-ctr, ctr) = (Wa @ f)[nbr] + ((Wb-Wa) @ f)[ctr].
    Node-level maps are computed once on PE, edge values gathered from DRAM
    by indirect DMA and transposed on PE to channels-major, so BN stats and
    k-max pooling are cheap per-partition ops.
  - BN batch stats all-reduced over all 8 cores; h1 all-gathered within each
    batch's 4-core group. LeakyReLU and the BN affine commute with k-max
    (positive scale), so they are applied after the reduction.
"""

import sys

sys.path.insert(0, "/opt/trn_rl_repo")

import numpy as np

import concourse.bass as bass
import concourse.tile as tile
from concourse import bacc, mybir
from concourse.bass import IndirectOffsetOnAxis
from concourse.bass_utils import run_bass_kernel_spmd
from concourse.masks import make_identity

F32 = mybir.dt.float32
BF16 = mybir.dt.bfloat16
U32 = mybir.dt.uint32
I32 = mybir.dt.int32
AX = mybir.AxisListType
OP = mybir.AluOpType
ACTF = mybir.ActivationFunctionType

B, N, C, D = 2, 4096, 64, 3
NS = 1024           # rows per core
QT = NS // 128      # 8 query tiles per core
PT = N // 128       # 32 point tiles
K1X, K1P, K2 = 8, 6, 32
K1 = K1X + K1P      # 14
C1 = 2 * C          # 128
HID = 128
OUTC = 64
EPS = 1e-5
SLOPE = 0.2
NEG = -1.0e30
M1 = float(B * N * K1)
M2 = float(B * N * K2)
NCORES = 8
TRACE = False       # set kernel.TRACE = True before calling for an NTFF profile
STUBS = set()       # timing-attribution stubs: {"coll","gather","topk","dist"}

_CACHE = {}


def _lrelu_inplace(nc, ap):
    # lrelu(x) = max(0.2*x, x), exact for slope in (0,1)
    nc.vector.scalar_tensor_tensor(out=ap, in0=ap, scalar=SLOPE, in1=ap,
                                   op0=OP.mult, op1=OP.max)


def _bn_coeffs(nc, pool, stat, gb, abn, M, tagp):
    """stat[:,0]=sum, stat[:,1]=sumsq (globally reduced) -> abn = [a, d]."""
    P = stat.shape[0]
    mu = pool.tile([P, 1], F32, tag=tagp + "mu")
    var = pool.tile([P, 1], F32, tag=tagp + "var")
    nc.vector.tensor_scalar_mul(mu[:], stat[:, 0:1], 1.0 / M)
    nc.vector.tensor_scalar_mul(var[:], stat[:, 1:2], 1.0 / M)
    mu2 = pool.tile([P, 1], F32, tag=tagp + "mu2")
    nc.vector.tensor_mul(mu2[:], mu[:], mu[:])
    nc.vector.tensor_sub(var[:], var[:], mu2[:])
    nc.vector.tensor_scalar_add(var[:], var[:], EPS)
    sd = pool.tile([P, 1], F32, tag=tagp + "sd")
    nc.scalar.activation(sd[:], var[:], ACTF.Sqrt)
    inv = pool.tile([P, 1], F32, tag=tagp + "inv")
    nc.vector.reciprocal(inv[:], sd[:])
    nc.vector.tensor_mul(abn[:, 0:1], gb[:, 0:1], inv[:])       # a = g/sd
    tmp = pool.tile([P, 1], F32, tag=tagp + "tmp")
    nc.vector.tensor_mul(tmp[:], mu[:], abn[:, 0:1])
    nc.vector.tensor_sub(abn[:, 1:2], gb[:, 1:2], tmp[:])       # d = b - mu*a


def _bcast_mid(ap2d, k):
    # (128, n) -> (128, k, n) with a step-0 broadcast middle dim
    return ap2d.rearrange("p (a n) -> p a n", a=1).to_broadcast(
        [ap2d.shape[0], k, ap2d.shape[1]])


def _build():
    nc = bacc.Bacc("TRN2", target_bir_lowering=False, debug=False,
                   num_devices=NCORES)

    t_xT = nc.dram_tensor("xT_full", [C, N], F32, kind="ExternalInput")
    t_pT = nc.dram_tensor("posT_full", [D, N], F32, kind="ExternalInput")
    t_xTl = nc.dram_tensor("xT_local", [C, NS], F32, kind="ExternalInput")
    t_pTl = nc.dram_tensor("posT_local", [D, NS], F32, kind="ExternalInput")
    t_xn = nc.dram_tensor("x_nat", [N, C], F32, kind="ExternalInput")
    t_pn = nc.dram_tensor("pos_nat", [N, D], F32, kind="ExternalInput")
    t_xnl = nc.dram_tensor("x_nat_local", [NS, C], F32, kind="ExternalInput")
    t_pnl = nc.dram_tensor("pos_nat_local", [NS, D], F32, kind="ExternalInput")
    t_u1 = nc.dram_tensor("U1T", [C, C1], F32, kind="ExternalInput")
    t_v1 = nc.dram_tensor("V1T", [C, C1], F32, kind="ExternalInput")
    t_u2 = nc.dram_tensor("U2T", [C1, HID], F32, kind="ExternalInput")
    t_v2 = nc.dram_tensor("V2T", [C1, HID], F32, kind="ExternalInput")
    t_w3 = nc.dram_tensor("W3T", [HID, OUTC], F32, kind="ExternalInput")
    t_gb1 = nc.dram_tensor("gb1", [C1, 2], F32, kind="ExternalInput")
    t_gb2 = nc.dram_tensor("gb2", [HID, 2], F32, kind="ExternalInput")
    t_gb3 = nc.dram_tensor("gb3", [OUTC, 2], F32, kind="ExternalInput")

    t_out = nc.dram_tensor("out_slice", [NS, OUTC], F32, kind="ExternalOutput")
    t_idx2 = nc.dram_tensor("idx2_slice", [NS, K2], I32, kind="ExternalOutput")

    io = dict(locals())
    with tile.TileContext(nc) as tc:
        _program(nc, tc, io)
    nc.compile()
    return nc


def _program(nc, tc, T):
    with (
        tc.tile_pool(name="persist", bufs=1) as persist,
        tc.tile_pool(name="dram", bufs=1, space="DRAM") as dram,
    ):
        ident = persist.tile([128, 128], F32)
        make_identity(nc, ident[:])
        ident_bf = persist.tile([128, 128], BF16)
        nc.vector.tensor_copy(ident_bf[:], ident[:])
        idx2_all = persist.tile([128, QT, K2], U32)
        h1_loc = persist.tile([C1, NS], F32)

        A_dram = dram.tile([N, C1], F32)
        P_dram = dram.tile([N, HID], BF16)
        nrow_x_d = dram.tile([1, N], F32)
        nrow_p_d = dram.tile([1, N], F32)
        ar1_in = dram.tile([C1, 2], F32)
        ar1_out = dram.tile([C1, 2], F32)
        ar2_in = dram.tile([HID, 2], F32)
        ar2_out = dram.tile([HID, 2], F32)
        ar3_in = dram.tile([OUTC, 2], F32)
        ar3_out = dram.tile([OUTC, 2], F32)
        ag_in = dram.tile([C1, NS], F32)
        ag_out = dram.tile([4, C1, NS], F32)

        _phase12(nc, tc, T, ident, idx2_all, h1_loc, A_dram,
                 nrow_x_d, nrow_p_d, ar1_in, ar1_out)
        _phase3(nc, tc, T, ident, ident_bf, idx2_all, h1_loc, P_dram,
                ag_in, ag_out, ar2_in, ar2_out, ar3_in, ar3_out)


def _phase12(nc, tc, T, ident, idx2_all, h1_loc, A_dram,
             nrow_x_d, nrow_p_d, ar1_in, ar1_out):
    with (
        tc.tile_pool(name="ph1", bufs=1) as p1,
        tc.tile_pool(name="ph1s", bufs=2) as p1s,
        tc.tile_pool(name="ph2", bufs=2) as p2,
        tc.tile_pool(name="stp", bufs=1) as stp,
    ):
        # packed transposed inputs: rows 0:64 = xT, rows 64:67 = posT
        xTpk = p1.tile([128, N], F32)
        xTlpk = p1.tile([128, NS], F32)
        nc.sync.dma_start(xTpk[0:C, :], T["t_xT"].ap())
        nc.sync.dma_start(xTpk[C:C + D, :], T["t_pT"].ap())
        nc.sync.dma_start(xTlpk[0:C, :], T["t_xTl"].ap())
        nc.sync.dma_start(xTlpk[C:C + D, :], T["t_pTl"].ap())

        # weights
        U1 = p1.tile([C, C1], F32)
        V1 = p1.tile([C, C1], F32)
        gb1 = p1.tile([C1, 2], F32)
        nc.sync.dma_start(U1[:], T["t_u1"].ap())
        nc.sync.dma_start(V1[:], T["t_v1"].ap())
        nc.sync.dma_start(gb1[:], T["t_gb1"].ap())

        # ---- norms (bit-exact: square + DVE reduce on natural layout) ----
        ncol_x = p1.tile([128, PT], F32)
        ncol_p = p1.tile([128, PT], F32)
        ni_x = p1.tile([128, QT], F32)
        ni_p = p1.tile([128, QT], F32)
        for i in range(PT):
            xt = p1s.tile([128, C], F32, tag="nx")
            nc.sync.dma_start(xt[:], T["t_xn"].ap()[i * 128:(i + 1) * 128, :])
            sq = p1s.tile([128, C], F32, tag="nsq")
            nc.vector.tensor_mul(sq[:], xt[:], xt[:])
            nc.vector.tensor_reduce(ncol_x[:, i:i + 1], sq[:], axis=AX.X,
                                    op=OP.add)
            pt_ = p1s.tile([128, D], F32, tag="np")
            nc.sync.dma_start(pt_[:], T["t_pn"].ap()[i * 128:(i + 1) * 128, :])
            sqp = p1s.tile([128, D], F32, tag="npsq")
            nc.vector.tensor_mul(sqp[:], pt_[:], pt_[:])
            nc.vector.tensor_reduce(ncol_p[:, i:i + 1], sqp[:], axis=AX.X,
                                    op=OP.add)
        for t in range(QT):
            xt = p1s.tile([128, C], F32, tag="nx")
            nc.sync.dma_start(xt[:], T["t_xnl"].ap()[t * 128:(t + 1) * 128, :])
            sq = p1s.tile([128, C], F32, tag="nsq")
            nc.vector.tensor_mul(sq[:], xt[:], xt[:])
            nc.vector.tensor_reduce(ni_x[:, t:t + 1], sq[:], axis=AX.X,
                                    op=OP.add)
            pt_ = p1s.tile([128, D], F32, tag="np")
            nc.sync.dma_start(pt_[:], T["t_pnl"].ap()[t * 128:(t + 1) * 128, :])
            sqp = p1s.tile([128, D], F32, tag="npsq")
            nc.vector.tensor_mul(sqp[:], pt_[:], pt_[:])
            nc.vector.tensor_reduce(ni_p[:, t:t + 1], sqp[:], axis=AX.X,
                                    op=OP.add)

        # (128, PT) -> flat (1, N) -> broadcast to 128 partitions
        nc.sync.dma_start(
            nrow_x_d[:].rearrange("a (t p) -> p (a t)", p=128), ncol_x[:])
        nc.sync.dma_start(
            nrow_p_d[:].rearrange("a (t p) -> p (a t)", p=128), ncol_p[:])
        nrow_x = p1.tile([1, N], F32)
        nrow_p = p1.tile([1, N], F32)
        nc.sync.dma_start(nrow_x[:], nrow_x_d[:])
        nc.sync.dma_start(nrow_p[:], nrow_p_d[:])
        # broadcast rows to 128 partitions via PE ones-matmul (x1.0 is exact;
        # gpsimd partition_broadcast is slow)
        Sx = p1.tile([128, N], F32)
        Sp = p1.tile([128, N], F32)
        ones_col = p1.tile([1, 128], F32)
        nc.gpsimd.memset(ones_col[:], 1.0)
        with tc.tile_pool(name="bcps", bufs=4, space="PSUM") as bcps:
            for j in range(N // 512):
                cs = slice(j * 512, (j + 1) * 512)
                ps = bcps.tile([128, 512], F32, tag="bc")
                nc.tensor.matmul(ps[:], ones_col[:], nrow_x[:, cs],
                                 start=True, stop=True)
                nc.scalar.copy(Sx[:, cs], ps[:])
                ps2 = bcps.tile([128, 512], F32, tag="bc")
                nc.tensor.matmul(ps2[:], ones_col[:], nrow_p[:, cs],
                                 start=True, stop=True)
                nc.scalar.copy(Sp[:, cs], ps2[:])

        # ---- node maps: A^T (row-major, DRAM) and B (channels-major) ----
        B_cm = p1.tile([C1, NS], F32)
        with (
            tc.tile_pool(name="aps", bufs=4, space="PSUM") as aps,
            tc.tile_pool(name="asb", bufs=4) as asb,
        ):
            for i in range(PT):
                ps = aps.tile([128, C1], F32, tag="aps")
                nc.tensor.matmul(ps[:], xTpk[0:C, i * 128:(i + 1) * 128],
                                 U1[:], start=True, stop=True)
                sb = asb.tile([128, C1], F32, tag="asb")
                nc.scalar.copy(sb[:], ps[:])
                nc.sync.dma_start(A_dram[:][i * 128:(i + 1) * 128, :], sb[:])
            for j in range(NS // 512):
                ps = aps.tile([C1, 512], F32, tag="bps")
                nc.tensor.matmul(ps[:], V1[:],
                                 xTlpk[0:C, j * 512:(j + 1) * 512],
                                 start=True, stop=True)
                nc.scalar.copy(B_cm[:, j * 512:(j + 1) * 512], ps[:])

        # ---- per-qtile: distances, top-k, layer-1 edge conv ----
        y1_all = p1.tile([128, QT, K1, 128], BF16)
        psum_s = p1.tile([C1, QT], F32)
        psum_q = p1.tile([C1, QT], F32)
        dnx = p1.tile([128, N], F32)
        dnp = p1.tile([128, N], F32)

        qtile_ps = tc.tile_pool(name="ph1ps", bufs=2, space="PSUM")
        g1ps_cm = tc.tile_pool(name="g1ps", bufs=1, space="PSUM")
        p1ps = qtile_ps.__enter__()
        g1ps = g1ps_cm.__enter__()
        for t in range(QT):
            # negated distances dneg = fl(2e - fl(n_i + n_j)) == -d_ref
            if "dist" in STUBS:
                nc.gpsimd.memset(dnx[:], -1.0)
                nc.gpsimd.memset(dnp[:], -1.0)
            for q in range(0 if "dist" in STUBS else 4):
                cs = slice(q * 1024, (q + 1) * 1024)
                sxc = p1s.tile([128, 1024], F32, tag="sxc")
                nc.vector.tensor_scalar_add(sxc[:], Sx[:, cs],
                                            ni_x[:, t:t + 1])
                ps = p1ps.tile([128, 1024], F32, tag="eps")
                for h in range(2):
                    c0 = q * 1024 + h * 512
                    nc.tensor.matmul(ps[:, h * 512:(h + 1) * 512],
                                     xTlpk[0:C, t * 128:(t + 1) * 128],
                                     xTpk[0:C, c0:c0 + 512],
                                     start=True, stop=True)
                nc.vector.scalar_tensor_tensor(
                    out=dnx[:, cs], in0=ps[:], scalar=2.0, in1=sxc[:],
                    op0=OP.mult, op1=OP.subtract)
                spc = p1s.tile([128, 1024], F32, tag="spc")
                nc.vector.tensor_scalar_add(spc[:], Sp[:, cs],
                                            ni_p[:, t:t + 1])
                ps2 = p1ps.tile([128, 1024], F32, tag="eps")
                for h in range(2):
                    c0 = q * 1024 + h * 512
                    nc.tensor.matmul(ps2[:, h * 512:(h + 1) * 512],
                                     xTlpk[C:C + D, t * 128:(t + 1) * 128],
                                     xTpk[C:C + D, c0:c0 + 512],
                                     start=True, stop=True)
                nc.vector.scalar_tensor_tensor(
                    out=dnp[:, cs], in0=ps2[:], scalar=2.0, in1=spc[:],
                    op0=OP.mult, op1=OP.subtract)

            # top-k: per-chunk max8 candidates, then full-row index finds
            if "topk" in STUBS:
                fx = p1.tile([128, 16], U32, tag="fx")
                fp = p1.tile([128, 40], U32, tag="fp")
                nc.gpsimd.memset(fx[:], 0)
                nc.gpsimd.memset(fp[:], 0)
            candx = p1.tile([128, 256], F32, tag="candx")
            candp = p1.tile([128, 256], F32, tag="candp")
            if "topk" not in STUBS:
              for ch in range(32):
                nc.vector.max(candx[:, ch * 8:(ch + 1) * 8],
                              dnx[:, ch * 128:(ch + 1) * 128])
            if "topk" not in STUBS:
              for ch in range(32):
                nc.vector.max(candp[:, ch * 8:(ch + 1) * 8],
                              dnp[:, ch * 128:(ch + 1) * 128])
              vx = p1.tile([128, 16], F32, tag="vx")
              fx = p1.tile([128, 16], U32, tag="fx")
              nc.vector.max(vx[:, 0:8], candx[:])
              nc.vector.max_index(fx[:, 0:8], vx[:, 0:8], dnx[:])
              nc.vector.match_replace(candx[:], vx[:, 0:8], candx[:], NEG)
              nc.vector.max(vx[:, 8:16], candx[:])
              nc.vector.max_index(fx[:, 8:16], vx[:, 8:16], dnx[:])

              vp = p1.tile([128, 40], F32, tag="vp")
              fp = p1.tile([128, 40], U32, tag="fp")
              for r in range(5):
                sl = slice(r * 8, (r + 1) * 8)
                nc.vector.max(vp[:, sl], candp[:])
                nc.vector.max_index(fp[:, sl], vp[:, sl], dnp[:])
                if r < 4:
                    nc.vector.match_replace(dnp[:], vp[:, sl], dnp[:], NEG)
                    nc.vector.match_replace(candp[:], vp[:, sl], candp[:],
                                            NEG)

            nc.vector.tensor_copy(idx2_all[:, t, :], fp[:, 1:33])
            idx2_i32 = p1.tile([128, K2], I32, tag="idx2i")
            nc.vector.tensor_copy(idx2_i32[:], fp[:, 1:33])
            nc.sync.dma_start(
                T["t_idx2"].ap()[t * 128:(t + 1) * 128, :], idx2_i32[:])

            # layer 1: gather A rows per k, transpose to channels-major, +B
            g1 = p2.tile([128, K1, C1], F32, tag="g1")
            for k in range(K1):
                if "gather" in STUBS:
                    nc.sync.dma_start(g1[:, k, :],
                                      A_dram[:][k * 128:(k + 1) * 128, :])
                    continue
                off = fx[:, 1 + k:2 + k] if k < K1X else fp[:, k - 7:k - 6]
                nc.gpsimd.indirect_dma_start(
                    out=g1[:, k, :], out_offset=None, in_=A_dram[:],
                    in_offset=IndirectOffsetOnAxis(ap=off, axis=0))
            yps = g1ps.tile([128, K1 * 128], F32, tag="yps")
            for k in range(K1):
                nc.tensor.transpose(yps[:, k * 128:(k + 1) * 128],
                                    g1[:, k, :], ident[:])
            nc.vector.scalar_tensor_tensor(
                out=y1_all[:, t, :, :],
                in0=yps[:].rearrange("p (k n) -> p k n", k=K1),
                scalar=0.0, op0=OP.add,
                in1=_bcast_mid(B_cm[:, t * 128:(t + 1) * 128], K1),
                op1=OP.add,
                accum_out=psum_s[:, t:t + 1])
            scr = p2.tile([128, K1 * 128], BF16, tag="scr1")
            nc.scalar.activation(
                scr[:], y1_all[:, t, :, :].rearrange("p a b -> p (a b)"),
                ACTF.Square, accum_out=psum_q[:, t:t + 1])

        g1ps_cm.__exit__(None, None, None)
        qtile_ps.__exit__(None, None, None)

        # ---- BN1 stats allreduce; h1 = lrelu(a*kmax(y1)+d) ----
        stat1 = stp.tile([C1, 2], F32)
        nc.vector.tensor_reduce(stat1[:, 0:1], psum_s[:], axis=AX.X, op=OP.add)
        nc.vector.tensor_reduce(stat1[:, 1:2], psum_q[:], axis=AX.X, op=OP.add)
        nc.sync.dma_start(ar1_in[:], stat1[:])
        if "coll" in STUBS:
            nc.sync.dma_start(ar1_out[:], ar1_in[:])
        else:
            nc.gpsimd.collective_compute(
                "AllReduce", OP.add, replica_groups=[list(range(NCORES))],
                ins=[ar1_in.opt()], outs=[ar1_out.opt()])
        nc.sync.dma_start(stat1[:], ar1_out[:])
        abn1 = stp.tile([C1, 2], F32)
        _bn_coeffs(nc, stp, stat1, gb1, abn1, M1, "b1")

        for t in range(QT):
            mx = p1s.tile([128, 128], F32, tag="mx1")
            nc.vector.tensor_reduce(
                mx[:], y1_all[:, t, :, :].rearrange("p k n -> p n k"),
                axis=AX.X, op=OP.max)
            nc.scalar.activation(h1_loc[:, t * 128:(t + 1) * 128], mx[:],
                                 ACTF.Identity, bias=abn1[:, 1:2],
                                 scale=abn1[:, 0:1])
            _lrelu_inplace(nc, h1_loc[:, t * 128:(t + 1) * 128])


def _phase3(nc, tc, T, ident, ident_bf, idx2_all, h1_loc, P_dram,
            ag_in, ag_out, ar2_in, ar2_out, ar3_in, ar3_out):
    with (
        tc.tile_pool(name="ph3", bufs=1) as p3,
        tc.tile_pool(name="st3", bufs=1) as stp,
    ):
        # allgather h1 within the 4-core batch group
        nc.sync.dma_start(ag_in[:], h1_loc[:])
        if "coll" in STUBS:
            for _s in range(4):
                nc.sync.dma_start(ag_out[:][_s, :, :], ag_in[:])
        else:
            nc.gpsimd.collective_compute(
                "AllGather", OP.bypass,
                replica_groups=[[0, 1, 2, 3], [4, 5, 6, 7]],
                ins=[ag_in.opt()], outs=[ag_out.opt()])

        U2 = p3.tile([C1, HID], F32)
        V2 = p3.tile([C1, HID], F32)
        W3 = p3.tile([HID, OUTC], F32)
        gb2 = p3.tile([HID, 2], F32)
        gb3 = p3.tile([OUTC, 2], F32)
        nc.sync.dma_start(U2[:], T["t_u2"].ap())
        nc.sync.dma_start(V2[:], T["t_v2"].ap())
        nc.sync.dma_start(W3[:], T["t_w3"].ap())
        nc.sync.dma_start(gb2[:], T["t_gb2"].ap())
        nc.sync.dma_start(gb3[:], T["t_gb3"].ap())

        y2b = p3.tile([HID, QT, K2, 128], BF16)
        s2s = p3.tile([HID, QT * 2], F32)
        s2q = p3.tile([HID, QT * 2], F32)

        with (
            tc.tile_pool(name="ph3a", bufs=1) as pa,
            tc.tile_pool(name="ph3a2", bufs=2) as pa2,
        ):
            h1f = pa.tile([C1, N], F32)
            for s in range(4):
                nc.sync.dma_start(h1f[:, s * NS:(s + 1) * NS],
                                  ag_out[:][s, :, :])
            # P map rows to DRAM; Q map channels-major (local rows)
            Q_cm = pa.tile([HID, NS], F32)
            with tc.tile_pool(name="pps", bufs=4, space="PSUM") as pps:
                for i in range(PT):
                    ps = pps.tile([128, HID], F32, tag="pps")
                    nc.tensor.matmul(ps[:], h1f[:, i * 128:(i + 1) * 128],
                                     U2[:], start=True, stop=True)
                    sb = pa2.tile([128, HID], BF16, tag="psb")
                    nc.scalar.copy(sb[:], ps[:])
                    nc.sync.dma_start(P_dram[:][i * 128:(i + 1) * 128, :],
                                      sb[:])
                for j in range(NS // 512):
                    ps = pps.tile([HID, 512], F32, tag="qps")
                    nc.tensor.matmul(ps[:], V2[:],
                                     h1_loc[:, j * 512:(j + 1) * 512],
                                     start=True, stop=True)
                    nc.scalar.copy(Q_cm[:, j * 512:(j + 1) * 512], ps[:])

            # layer 2 per qtile: gather P rows, transpose, +Q -> y2 (bf16)
            g2ps_cm = tc.tile_pool(name="g2ps", bufs=2, space="PSUM")
            g2ps = g2ps_cm.__enter__()
            for t in range(QT):
                g2 = pa2.tile([128, K2, HID], BF16, tag="g2")
                for k in range(K2):
                    if "gather" in STUBS:
                        nc.sync.dma_start(g2[:, k, :],
                                          P_dram[:][k * 128:(k + 1) * 128, :])
                        continue
                    nc.gpsimd.indirect_dma_start(
                        out=g2[:, k, :], out_offset=None, in_=P_dram[:],
                        in_offset=IndirectOffsetOnAxis(
                            ap=idx2_all[:, t, k:k + 1], axis=0))
                for half in range(2):
                    ps = g2ps.tile([128, 16 * 128], BF16, tag="g2ps")
                    for k in range(16):
                        kk = half * 16 + k
                        nc.tensor.transpose(ps[:, k * 128:(k + 1) * 128],
                                            g2[:, kk, :], ident_bf[:])
                    col = t * 2 + half
                    nc.vector.scalar_tensor_tensor(
                        out=y2b[:, t, half * 16:(half + 1) * 16, :],
                        in0=ps[:].rearrange("p (k n) -> p k n", k=16),
                        scalar=0.0, op0=OP.add,
                        in1=_bcast_mid(Q_cm[:, t * 128:(t + 1) * 128], 16),
                        op1=OP.add,
                        accum_out=s2s[:, col:col + 1])
                    scr = pa2.tile([128, 16 * 128], BF16, tag="scr2")
                    nc.scalar.activation(
                        scr[:],
                        y2b[:, t, half * 16:(half + 1) * 16, :].rearrange(
                            "p a b -> p (a b)"),
                        ACTF.Square, accum_out=s2q[:, col:col + 1])
            g2ps_cm.__exit__(None, None, None)

        stat2 = stp.tile([HID, 2], F32)
        nc.vector.tensor_reduce(stat2[:, 0:1], s2s[:], axis=AX.X, op=OP.add)
        nc.vector.tensor_reduce(stat2[:, 1:2], s2q[:], axis=AX.X, op=OP.add)
        nc.sync.dma_start(ar2_in[:], stat2[:])
        if "coll" in STUBS:
            nc.sync.dma_start(ar2_out[:], ar2_in[:])
        else:
            nc.gpsimd.collective_compute(
                "AllReduce", OP.add, replica_groups=[list(range(NCORES))],
                ins=[ar2_in.opt()], outs=[ar2_out.opt()])
        nc.sync.dma_start(stat2[:], ar2_out[:])
        abn2 = stp.tile([HID, 2], F32)
        _bn_coeffs(nc, stp, stat2, gb2, abn2, M2, "b2")

        # ---- layer 3: h2 = lrelu(a2*y2+d2); y3 = W3 @ h2; stats + k-max ----
        s3s = stp.tile([128, QT // 2], F32)
        s3q = stp.tile([128, QT // 2], F32)
        mx_all = stp.tile([128, (QT // 2) * 128], F32)
        with (
            tc.tile_pool(name="ph3b", bufs=2) as pb,
            tc.tile_pool(name="y3ps", bufs=4, space="PSUM") as y3ps,
        ):
            for pair in range(QT // 2):
                h2s = []
                for half in range(2):
                    t = pair * 2 + half
                    h2 = pb.tile([HID, K2 * 128], F32, tag=f"h2{half}")
                    nc.scalar.activation(
                        h2[:],
                        y2b[:, t, :, :].rearrange("p a b -> p (a b)"),
                        ACTF.Identity, bias=abn2[:, 1:2], scale=abn2[:, 0:1])
                    _lrelu_inplace(nc, h2[:])
                    h2s.append(h2)
                y3 = pb.tile([128, K2 * 128], F32, tag="y3")
                nchunk = K2 * 128 // 512
                s3c = pb.tile([128, nchunk], F32, tag="s3c")
                for q in range(nchunk):
                    cs = slice(q * 512, (q + 1) * 512)
                    ps = y3ps.tile([128, 512], F32, tag="y3ps")
                    nc.tensor.matmul(ps[0:OUTC, :], W3[:], h2s[0][:, cs],
                                     start=True, stop=True)
                    nc.tensor.matmul(ps[OUTC:128, :], W3[:], h2s[1][:, cs],
                                     start=True, stop=True)
                    nc.scalar.activation(y3[:, cs], ps[:], ACTF.Copy,
                                         accum_out=s3c[:, q:q + 1])
                col = slice(pair, pair + 1)
                nc.vector.tensor_reduce(s3s[:, col], s3c[:], axis=AX.X,
                                        op=OP.add)
                scr = pb.tile([128, K2 * 128], BF16, tag="scr3")
                nc.scalar.activation(scr[:], y3[:], ACTF.Square,
                                     accum_out=s3q[:, col])
                nc.vector.tensor_reduce(
                    mx_all[:, pair * 128:(pair + 1) * 128],
                    y3[:].rearrange("p (k n) -> p n k", k=K2),
                    axis=AX.X, op=OP.max)

        # fold packed halves (cross-partition via DMA), allreduce, coeffs
        s3s_f = stp.tile([128, 1], F32)
        s3q_f = stp.tile([128, 1], F32)
        nc.vector.tensor_reduce(s3s_f[:], s3s[:], axis=AX.X, op=OP.add)
        nc.vector.tensor_reduce(s3q_f[:], s3q[:], axis=AX.X, op=OP.add)
        hi = stp.tile([OUTC, 2], F32)
        nc.sync.dma_start(hi[:, 0:1], s3s_f[OUTC:128, :])
        nc.sync.dma_start(hi[:, 1:2], s3q_f[OUTC:128, :])
        stat3 = stp.tile([OUTC, 2], F32)
        nc.vector.tensor_add(stat3[:, 0:1], s3s_f[0:OUTC, :], hi[:, 0:1])
        nc.vector.tensor_add(stat3[:, 1:2], s3q_f[0:OUTC, :], hi[:, 1:2])
        nc.sync.dma_start(ar3_in[:], stat3[:])
        if "coll" in STUBS:
            nc.sync.dma_start(ar3_out[:], ar3_in[:])
        else:
            nc.gpsimd.collective_compute(
                "AllReduce", OP.add, replica_groups=[list(range(NCORES))],
                ins=[ar3_in.opt()], outs=[ar3_out.opt()])
        nc.sync.dma_start(stat3[:], ar3_out[:])
        abn3_64 = stp.tile([OUTC, 2], F32)
        _bn_coeffs(nc, stp, stat3, gb3, abn3_64, M2, "b3")
        abn3 = stp.tile([128, 2], F32)
        nc.vector.tensor_copy(abn3[0:OUTC, :], abn3_64[:])
        nc.sync.dma_start(abn3[OUTC:128, :], abn3_64[:])

        # ---- out = lrelu(a3 * kmax(y3) + d3), transpose, store ----
        with (
            tc.tile_pool(name="oute", bufs=2) as po,
            tc.tile_pool(name="outps", bufs=2, space="PSUM") as ops_,
        ):
            for pair in range(QT // 2):
                af = po.tile([128, 128], F32, tag="afo")
                nc.scalar.activation(
                    af[:], mx_all[:, pair * 128:(pair + 1) * 128],
                    ACTF.Identity, bias=abn3[:, 1:2], scale=abn3[:, 0:1])
                _lrelu_inplace(nc, af[:])
                tp = ops_.tile([128, 128], F32, tag="tpo")
                nc.tensor.transpose(tp[:], af[:], ident[:])
                ot = po.tile([128, 128], F32, tag="oto")
                nc.scalar.copy(ot[:], tp[:])
                r0 = pair * 256
                nc.sync.dma_start(T["t_out"].ap()[r0:r0 + 128, :],
                                  ot[:, 0:OUTC])
                nc.sync.dma_start(T["t_out"].ap()[r0 + 128:r0 + 256, :],
                                  ot[:, OUTC:128])


def build_replicated(reps):
    """Build a program with the kernel body replicated `reps` times
    (for device-time measurement by wall-clock delta vs reps=1)."""
    nc = bacc.Bacc("TRN2", target_bir_lowering=False, debug=False,
                   num_devices=NCORES)
    t_xT = nc.dram_tensor("xT_full", [C, N], F32, kind="ExternalInput")
    t_pT = nc.dram_tensor("posT_full", [D, N], F32, kind="ExternalInput")
    t_xTl = nc.dram_tensor("xT_local", [C, NS], F32, kind="ExternalInput")
    t_pTl = nc.dram_tensor("posT_local", [D, NS], F32, kind="ExternalInput")
    t_xn = nc.dram_tensor("x_nat", [N, C], F32, kind="ExternalInput")
    t_pn = nc.dram_tensor("pos_nat", [N, D], F32, kind="ExternalInput")
    t_xnl = nc.dram_tensor("x_nat_local", [NS, C], F32, kind="ExternalInput")
    t_pnl = nc.dram_tensor("pos_nat_local", [NS, D], F32, kind="ExternalInput")
    t_u1 = nc.dram_tensor("U1T", [C, C1], F32, kind="ExternalInput")
    t_v1 = nc.dram_tensor("V1T", [C, C1], F32, kind="ExternalInput")
    t_u2 = nc.dram_tensor("U2T", [C1, HID], F32, kind="ExternalInput")
    t_v2 = nc.dram_tensor("V2T", [C1, HID], F32, kind="ExternalInput")
    t_w3 = nc.dram_tensor("W3T", [HID, OUTC], F32, kind="ExternalInput")
    t_gb1 = nc.dram_tensor("gb1", [C1, 2], F32, kind="ExternalInput")
    t_gb2 = nc.dram_tensor("gb2", [HID, 2], F32, kind="ExternalInput")
    t_gb3 = nc.dram_tensor("gb3", [OUTC, 2], F32, kind="ExternalInput")
    t_out = nc.dram_tensor("out_slice", [NS, OUTC], F32, kind="ExternalOutput")
    t_idx2 = nc.dram_tensor("idx2_slice", [NS, K2], I32, kind="ExternalOutput")
    io = dict(locals())
    with tile.TileContext(nc) as tc:
        for _ in range(reps):
            _program(nc, tc, io)
    nc.compile()
    return nc


def timed_exec(iters=20, nc=None, pipelined=True):
    """Run the kernel executable with device-resident inputs. With
    pipelined=True, dispatch `iters` calls asynchronously and divide the
    total wall by iters (amortizes the ~3.3ms axon relay floor).
    Returns (min_or_piped_s, mean_s)."""
    import time

    import jax
    from jax.sharding import Mesh, PartitionSpec
    from jax.experimental.shard_map import shard_map

    from concourse import bass2jax, mybir

    if nc is None:
        nc = _CACHE["nc"]
    in_maps = _CACHE["last_in_maps"]
    n_cores = NCORES

    partition_name = (nc.partition_id_tensor.name
                      if nc.partition_id_tensor else None)
    in_names, out_names, out_avals, zero_outs = [], [], [], []
    for alloc in nc.m.functions[0].allocations:
        if not isinstance(alloc, mybir.MemoryLocationSet):
            continue
        name = alloc.memorylocations[0].name
        if alloc.kind == "ExternalInput":
            if name != partition_name:
                in_names.append(name)
        elif alloc.kind == "ExternalOutput":
            out_names.append(name)
            out_avals.append(jax.core.ShapedArray(
                tuple(alloc.tensor_shape), mybir.dt.np(alloc.dtype)))
            zero_outs.append(np.zeros(tuple(alloc.tensor_shape),
                                      mybir.dt.np(alloc.dtype)))
    n_params = len(in_names)
    all_names = in_names + out_names
    if partition_name is not None:
        all_names.append(partition_name)

    def _body(*args):
        operands = list(args)
        if partition_name is not None:
            operands.append(bass2jax.partition_id_tensor())
        outs = bass2jax._bass_exec_p.bind(
            *operands,
            out_avals=tuple(out_avals),
            in_names=tuple(all_names),
            out_names=tuple(out_names),
            lowering_input_output_aliases=(),
            sim_require_finite=True,
            sim_require_nnan=True,
            nc=nc,
        )
        return tuple(outs)

    devices = jax.devices()[:n_cores]
    mesh = Mesh(np.asarray(devices), ("core",))
    nin = n_params + len(out_names)
    fn = jax.jit(
        shard_map(_body, mesh=mesh,
                  in_specs=(PartitionSpec("core"),) * nin,
                  out_specs=(PartitionSpec("core"),) * len(out_names),
                  check_rep=False),
        keep_unused=True,
    )
    from jax.sharding import NamedSharding
    sh = NamedSharding(mesh, PartitionSpec("core"))
    args = []
    for i, name in enumerate(in_names):
        cat = np.concatenate([np.asarray(m[name]) for m in in_maps], axis=0)
        args.append(jax.device_put(cat, sh))
    for z in zero_outs:
        cat = np.zeros((n_cores * z.shape[0], *z.shape[1:]), z.dtype)
        args.append(jax.device_put(cat, sh))
    # warmup + compile
    outs = fn(*args)
    jax.block_until_ready(outs)
    if pipelined:
        t0 = time.perf_counter()
        outs_l = [fn(*args) for _ in range(iters)]
        jax.block_until_ready(outs_l)
        piped = (time.perf_counter() - t0) / iters
        return piped, piped
    times = []
    for _ in range(iters):
        t0 = time.perf_counter()
        outs = fn(*args)
        jax.block_until_ready(outs)
        times.append(time.perf_counter() - t0)
    return min(times), sum(times) / len(times)


def kernel(x, pos, W1, g1, b1, W2, g2, b2, W3, g3, b3):
    x = np.ascontiguousarray(np.asarray(x, dtype=np.float32))
    pos = np.ascontiguousarray(np.asarray(pos, dtype=np.float32))
    W1 = np.asarray(W1, np.float32); W2 = np.asarray(W2, np.float32)
    W3 = np.asarray(W3, np.float32)
    g1 = np.asarray(g1, np.float32); b1 = np.asarray(b1, np.float32)
    g2 = np.asarray(g2, np.float32); b2 = np.asarray(b2, np.float32)
    g3 = np.asarray(g3, np.float32); b3 = np.asarray(b3, np.float32)

    if "nc" not in _CACHE:
        _CACHE["nc"] = _build()
    nc = _CACHE["nc"]

    U1T = np.ascontiguousarray(W1[:, :C].T)
    V1T = np.ascontiguousarray((W1[:, C:] - W1[:, :C]).T)
    U2T = np.ascontiguousarray(W2[:, :C1].T)
    V2T = np.ascontiguousarray((W2[:, C1:] - W2[:, :C1]).T)
    W3T = np.ascontiguousarray(W3.T)
    gb1 = np.ascontiguousarray(np.stack([g1, b1], axis=1))
    gb2 = np.ascontiguousarray(np.stack([g2, b2], axis=1))
    gb3 = np.ascontiguousarray(np.stack([g3, b3], axis=1))

    in_maps = []
    for c in range(NCORES):
        b, s = divmod(c, 4)
        sl = slice(s * NS, (s + 1) * NS)
        xT = np.ascontiguousarray(x[b].T)
        pT = np.ascontiguousarray(pos[b].T)
        in_maps.append({
            "xT_full": xT, "posT_full": pT,
            "xT_local": np.ascontiguousarray(xT[:, sl]),
            "posT_local": np.ascontiguousarray(pT[:, sl]),
            "x_nat": x[b], "pos_nat": pos[b],
            "x_nat_local": np.ascontiguousarray(x[b][sl]),
            "pos_nat_local": np.ascontiguousarray(pos[b][sl]),
            "U1T": U1T, "V1T": V1T, "U2T": U2T, "V2T": V2T, "W3T": W3T,
            "gb1": gb1, "gb2": gb2, "gb3": gb3,
        })

    _CACHE["last_in_maps"] = in_maps
    res = run_bass_kernel_spmd(nc, in_maps, core_ids=list(range(NCORES)),
                               trace=TRACE)
    _CACHE["last_results"] = res

    out = np.empty((B, N, OUTC), np.float32)
    idx2 = np.empty((B, N, K2), np.int32)
    for c in range(NCORES):
        b, s = divmod(c, 4)
        sl = slice(s * NS, (s + 1) * NS)
        out[b, sl] = res.results[c]["out_slice"]
        idx2[b, sl] = res.results[c]["idx2_slice"]
    return out, idx2
